# revision 1
# baseline (speedup 1.0000x reference)
"""Trainium2 Bass kernel for nn_Decoder (MusicVAE-style hierarchical decoder).

Strategy (8 NeuronCores, data-parallel over batch, no inter-core comms):
  - Conductor LSTM (16 sequential levels, batch 32/core) computes per-level
    embeddings.
  - Decoder levels are INDEPENDENT (initial state from dec_h0/dec_c0,
    note0=0), so all 16 levels are batched: effective decoder batch
    16*32 = 512 rows per core, 16 sequential note steps.
  - The conductor embedding is constant within a level, so its gate
    contribution (emb @ Wih[:, :H].T + bias) is precomputed once ("ge").
  - Everything lives feature-major: [features on partitions, rows free].
    Weights are the stationary matmul operand, activations stream.
  - bf16 matmuls with fp32 PSUM accumulation; c state in fp32.
"""
import numpy as np
import ml_dtypes

import concourse.bacc as bacc
import concourse.tile as tile
import concourse.mybir as mybir
from concourse.bass_utils import run_bass_kernel_spmd

bf16 = ml_dtypes.bfloat16
F32 = mybir.dt.float32
BF = mybir.dt.bfloat16
AF = mybir.ActivationFunctionType

NCORES = 8
B, Z, H, T = 256, 512, 1024, 512
L, NS = 16, 16
Bc = B // NCORES            # 32 batch rows per core
R = L * Bc                  # 512 decoder rows per core (levels x batch)
HK, TK, ZK = H // 128, T // 128, Z // 128   # 8, 4, 4
G = 4 * H // 128            # 32 gate chunks of 128


def _declare(nc):
    d = {}
    ei = dict(kind="ExternalInput")
    d["ones"] = nc.dram_tensor("ones", [1, R], BF, **ei)
    d["cbias"] = nc.dram_tensor("cbias", [1, 4 * H], BF, **ei)
    d["dbias"] = nc.dram_tensor("dbias", [1, 4 * H], BF, **ei)
    d["obias"] = nc.dram_tensor("obias", [128, TK], F32, **ei)
    d["zT"] = nc.dram_tensor("zT", [128, ZK, R], BF, **ei)
    d["h0T"] = nc.dram_tensor("h0T", [128, HK, R], BF, **ei)
    d["c0T"] = nc.dram_tensor("c0T", [128, HK, R], F32, **ei)
    d["cwih"] = nc.dram_tensor("cwih", [128, ZK, 4 * H], BF, **ei)
    d["cwhh"] = nc.dram_tensor("cwhh", [128, HK, 4 * H], BF, **ei)
    d["dwe"] = nc.dram_tensor("dwe", [G, 128, HK * 128], BF, **ei)
    d["dwn"] = nc.dram_tensor("dwn", [128, TK, 4 * H], BF, **ei)
    d["dwhh"] = nc.dram_tensor("dwhh", [128, HK, 4 * H], BF, **ei)
    d["owt"] = nc.dram_tensor("owt", [128, HK, T], BF, **ei)
    d["outbuf"] = nc.dram_tensor("outbuf", [NS, TK, 128, R], BF,
                                 kind="ExternalOutput")
    return d


PHASE_MARKS = []
_STRIP = []


def _mm(nc, out, w, x, start, stop):
    """Matmul; optionally emitted as standalone LDWEIGHTS + stripped matmul
    so the PE reorder window overlaps the weight load with the previous
    matmul's streaming."""
    if SPLIT_LDW:
        nc.tensor.ldweights(w)
        bi = nc.tensor.matmul(out, w, x, start=start, stop=stop)
        _STRIP.append(bi.ins)
        return bi
    return nc.tensor.matmul(out, w, x, start=start, stop=stop)


def _mark(nc, name):
    try:
        PHASE_MARKS.append((name, len(nc.all_instructions())))
    except Exception:
        pass


def _body(nc, tc, d):
    import contextlib
    with contextlib.ExitStack() as ctx:
        Pp = ctx.enter_context(tc.tile_pool(name="persist", bufs=1))

        t_ones = Pp.tile([1, R], BF, tag="ones")
        nc.sync.dma_start(t_ones[:], d["ones"][:])
        t_ob = Pp.tile([128, TK], F32, tag="obias")
        nc.sync.dma_start(t_ob[:], d["obias"][:])
        t_emb = Pp.tile([128, HK, R], BF, tag="emb")
        t_h = [Pp.tile([128, HK, R], BF, tag=f"hT{i}", name=f"hT{i}")
               for i in (0, 1)]
        t_c = Pp.tile([128, HK, R], F32, tag="c")
        t_note = Pp.tile([128, TK, R], BF, tag="note")
        nc.gpsimd.dma_start(t_h[0][:], d["h0T"][:])
        nc.gpsimd.dma_start(t_c[:], d["c0T"][:])

        # ---------------- conductor ----------------
        with tc.tile_pool(name="cond", bufs=1) as Pc, \
             tc.tile_pool(name="ctmp", bufs=2) as Pt, \
             tc.tile_pool(name="cps", bufs=4, space="PSUM") as PSc, \
             tc.tile_pool(name="gzps", bufs=2, space="PSUM") as PSz:
            t_cb = Pc.tile([1, 4 * H], BF, tag="cbias")
            nc.sync.dma_start(t_cb[:], d["cbias"][:])
            t_cwih = Pc.tile([128, ZK, 4 * H], BF, tag="cwih")
            nc.sync.dma_start(t_cwih[:], d["cwih"][:])
            t_zT = Pc.tile([128, ZK, R], BF, tag="zT")
            nc.sync.dma_start(t_zT[:], d["zT"][:])
            t_cwhh = Pc.tile([128, HK, 4 * H], BF, tag="cwhh")
            nc.sync.dma_start(t_cwhh[:], d["cwhh"][:])
            t_gz = Pc.tile([128, G, R], BF, tag="gz")
            t_cc = Pc.tile([128, HK, Bc], F32, tag="cc")

            # gz = z @ cond_Wih.T + cond_b for all levels at once
            _mark(nc, "gz")
            for m in range(G):
                ms = slice(m * 128, (m + 1) * 128)
                ps = PSz.tile([128, R], F32, tag="gzp")
                _mm(nc, ps[:], t_cb[0:1, ms], t_ones[:], True, False)
                for k in range(ZK):
                    _mm(nc, ps[:], t_cwih[:, k, ms], t_zT[:, k, :], False, (k == ZK - 1))
                nc.vector.tensor_copy(t_gz[:, m, :], ps[:])

            # sequential levels
            _mark(nc, "conductor")
            for _crep in range(COND_REPS):
              for lv in range(L):
                  cs = slice(lv * Bc, (lv + 1) * Bc)
                  ps_prev = slice((lv - 1) * Bc, lv * Bc)
                  for p in range(HK):
                      mi, mf, mg, mo = p, HK + p, 2 * HK + p, 3 * HK + p
                      ti = Pt.tile([128, Bc], BF, tag="ti")
                      tg = Pt.tile([128, Bc], BF, tag="tg")
                      to = Pt.tile([128, Bc], BF, tag="to")
                      tcn = Pt.tile([128, Bc], BF, tag="tcn")
                      tm1 = Pt.tile([128, Bc], BF, tag="tm1")
                      if lv == 0:
                          # h0 == 0: gates are just gz; c0 == 0: c = sig(i)*tanh(g)
                          nc.scalar.activation(ti[:], t_gz[:, mi, cs], AF.Sigmoid)
                          nc.scalar.activation(tg[:], t_gz[:, mg, cs], AF.Tanh)
                          nc.scalar.activation(to[:], t_gz[:, mo, cs], AF.Sigmoid)
                          nc.vector.tensor_mul(t_cc[:, p, :], ti[:], tg[:])
                      else:
                          ps = PSc.tile([128, 4, Bc], F32, tag="cgp")
                          for gi, m in enumerate((mi, mf, mg, mo)):
                              ms = slice(m * 128, (m + 1) * 128)
                              for k in range(HK):
                                  _mm(nc, ps[:, gi, :], t_cwhh[:, k, ms],
                                      t_emb[:, k, ps_prev], (k == 0), (k == HK - 1))
                          tf = Pt.tile([128, Bc], BF, tag="tf")
                          tm2 = Pt.tile([128, Bc], F32, tag="tm2")
                          gsi = Pt.tile([128, Bc], BF, tag="gsi")
                          gsf = Pt.tile([128, Bc], BF, tag="gsf")
                          gsg = Pt.tile([128, Bc], BF, tag="gsg")
                          gso = Pt.tile([128, Bc], BF, tag="gso")
                          nc.vector.tensor_add(gsi[:], ps[:, 0, :], t_gz[:, mi, cs])
                          nc.vector.tensor_add(gsf[:], ps[:, 1, :], t_gz[:, mf, cs])
                          nc.vector.tensor_add(gsg[:], ps[:, 2, :], t_gz[:, mg, cs])
                          nc.vector.tensor_add(gso[:], ps[:, 3, :], t_gz[:, mo, cs])
                          nc.scalar.activation(ti[:], gsi[:], AF.Sigmoid)
                          nc.scalar.activation(tf[:], gsf[:], AF.Sigmoid)
                          nc.scalar.activation(tg[:], gsg[:], AF.Tanh)
                          nc.scalar.activation(to[:], gso[:], AF.Sigmoid)
                          nc.vector.tensor_mul(tm1[:], ti[:], tg[:])
                          nc.vector.tensor_mul(tm2[:], tf[:], t_cc[:, p, :])
                          nc.vector.tensor_add(t_cc[:, p, :], tm1[:], tm2[:])
                      nc.scalar.activation(tcn[:], t_cc[:, p, :], AF.Tanh)
                      nc.vector.tensor_mul(t_emb[:, p, cs], to[:], tcn[:])


        # ge persists through the decoder (allocated after conductor frees)
        Pge = ctx.enter_context(tc.tile_pool(name="gepool", bufs=1))
        t_ge = Pge.tile([128, G, R], BF, tag="ge")

        # decoder weights (bulk, SWDGE queues; overlap with ge phase)
        Pw = ctx.enter_context(tc.tile_pool(name="wdec", bufs=1))
        t_dwn = Pw.tile([128, TK, 4 * H], BF, tag="dwn")
        nc.gpsimd.dma_start(t_dwn[:], d["dwn"][:])
        t_dwhh = Pw.tile([128, HK, 4 * H], BF, tag="dwhh")
        nc.gpsimd.dma_start(t_dwhh[:], d["dwhh"][:])
        t_owt = Pw.tile([128, HK, T], BF, tag="owt")
        nc.gpsimd.dma_start(t_owt[:], d["owt"][:])

        # ---------------- ge = emb @ dec_Wih[:, :H].T + dec_b ----------------
        _mark(nc, "ge")
        with tc.tile_pool(name="gew", bufs=4) as Pgw, \
             tc.tile_pool(name="geps", bufs=2, space="PSUM") as PSg:
            t_dbias = Pgw.tile([1, 4 * H], BF, tag="dbias", bufs=1)
            nc.sync.dma_start(t_dbias[:], d["dbias"][:])
            for m in range(G):
                ms = slice(m * 128, (m + 1) * 128)
                wt = Pgw.tile([128, HK, 128], BF, tag="dwe")
                nc.sync.dma_start(wt[:], d["dwe"][m].rearrange(
                    "p (k j) -> p k j", k=HK))
                ps = PSg.tile([128, R], F32, tag="gep")
                _mm(nc, ps[:], t_dbias[0:1, ms], t_ones[:], True, False)
                for k in range(HK):
                    _mm(nc, ps[:], wt[:, k, :], t_emb[:, k, :], False, (k == HK - 1))
                nc.vector.tensor_copy(t_ge[:, m, :], ps[:])

        # ---------------- decoder: 16 note steps over 512 rows --------------
        with tc.tile_pool(name="dtmp", bufs=2) as Pdt, \
             tc.tile_pool(name="dps", bufs=3, space="PSUM") as PSd, \
             tc.tile_pool(name="dpso", bufs=2, space="PSUM") as PSo:
            for _drep in range(DEC_REPS):
              for t in range(NS):
                  _mark(nc, f"dec{t:02d}")
                  hin = t_h[t % 2]
                  hout = t_h[(t + 1) % 2]
                  for p in range(HK):
                      psA = PSd.tile([128, 2, R], F32, tag="dgp", name="psA")
                      psB = PSd.tile([128, 2, R], F32, tag="dgp", name="psB")
                      for gi, m in enumerate((p, HK + p, 2 * HK + p,
                                              3 * HK + p)):
                          pst = psA if gi < 2 else psB
                          sl = gi % 2
                          ms = slice(m * 128, (m + 1) * 128)
                          for k in range(HK):
                              _mm(nc, pst[:, sl, :], t_dwhh[:, k, ms],
                                  hin[:, k, :], (k == 0), (k == HK - 1 and t == 0))
                          if t > 0:
                              for k in range(TK):
                                  _mm(nc, pst[:, sl, :], t_dwn[:, k, ms],
                                      t_note[:, k, :], False, (k == TK - 1))
                      gsi = Pdt.tile([128, R], BF, tag="gsi")
                      gsf = Pdt.tile([128, R], BF, tag="gsf")
                      gsg = Pdt.tile([128, R], BF, tag="gsg")
                      gso = Pdt.tile([128, R], BF, tag="gso")
                      nc.vector.tensor_add(gsi[:], psA[:, 0, :], t_ge[:, p, :])
                      nc.vector.tensor_add(gsf[:], psA[:, 1, :],
                                           t_ge[:, HK + p, :])
                      nc.vector.tensor_add(gsg[:], psB[:, 0, :],
                                           t_ge[:, 2 * HK + p, :])
                      nc.vector.tensor_add(gso[:], psB[:, 1, :],
                                           t_ge[:, 3 * HK + p, :])
                      ti = Pdt.tile([128, R], BF, tag="ti")
                      tf = Pdt.tile([128, R], BF, tag="tf")
                      tg = Pdt.tile([128, R], BF, tag="tg")
                      to = Pdt.tile([128, R], BF, tag="to")
                      tcn = Pdt.tile([128, R], BF, tag="tcn")
                      tm1 = Pdt.tile([128, R], BF, tag="tm1")
                      tm2 = Pdt.tile([128, R], F32, tag="tm2")
                      nc.scalar.activation(ti[:], gsi[:], AF.Sigmoid)
                      nc.scalar.activation(tf[:], gsf[:], AF.Sigmoid)
                      nc.scalar.activation(tg[:], gsg[:], AF.Tanh)
                      nc.scalar.activation(to[:], gso[:], AF.Sigmoid)
                      nc.vector.tensor_mul(tm1[:], ti[:], tg[:])
                      nc.vector.tensor_mul(tm2[:], tf[:], t_c[:, p, :])
                      nc.vector.tensor_add(t_c[:, p, :], tm1[:], tm2[:])
                      nc.scalar.activation(tcn[:], t_c[:, p, :], AF.Tanh)
                      nc.vector.tensor_mul(hout[:, p, :], to[:], tcn[:])
                  # output projection + sigmoid -> note (bf16, also the output)
                  _mark(nc, f"oproj{t:02d}")
                  for tk in range(TK):
                      ts_ = slice(tk * 128, (tk + 1) * 128)
                      po = PSo.tile([128, R], F32, tag="dpo")
                      for k in range(HK):
                          _mm(nc, po[:], t_owt[:, k, ts_],
                                           hout[:, k, :], (k == 0), (k == HK - 1))
                      nc.scalar.activation(t_note[:, tk, :], po[:],
                                           AF.Sigmoid, bias=t_ob[:, tk:tk + 1])
                      nc.sync.dma_start(d["outbuf"][t, tk], t_note[:, tk, :])




import os
DEC_REPS = int(os.environ.get("KBENCH_DEC_REPS", "1"))
SPLIT_LDW = os.environ.get("KBENCH_SPLIT_LDW", "0") == "1"
COND_REPS = int(os.environ.get("KBENCH_COND_REPS", "1"))

_CACHE = {}


def _build():
    if "nc" not in _CACHE:
        nc = bacc.Bacc("TRN2", target_bir_lowering=False, debug=False,
                       num_devices=NCORES)
        d = _declare(nc)
        _STRIP.clear()
        with tile.TileContext(nc) as tc:
            _body(nc, tc, d)
        for inst in _STRIP:
            inst.ins = [inst.ins[0]]
        nc.compile()
        _CACHE["nc"] = nc
    return _CACHE["nc"]


def _feat_major(W):
    """[J, K] -> [128, K/128, J] bf16 (stationary lhsT chunk layout)."""
    J, K = W.shape
    return np.ascontiguousarray(
        W.reshape(J, K // 128, 128).transpose(2, 1, 0)).astype(bf16)


def _pack_inputs(inputs):
    z = np.asarray(inputs["z"], np.float32)
    dec_h0 = np.asarray(inputs["dec_h0"], np.float32)
    dec_c0 = np.asarray(inputs["dec_c0"], np.float32)
    cond_b = np.asarray(inputs["cond_bih"] + inputs["cond_bhh"], np.float32)
    dec_b = np.asarray(inputs["dec_bih"] + inputs["dec_bhh"], np.float32)
    out_b = np.asarray(inputs["out_b"], np.float32)

    shared = {
        "ones": np.ones((1, R), dtype=bf16),
        "cbias": cond_b[None, :].astype(bf16),
        "dbias": dec_b[None, :].astype(bf16),
        "obias": np.ascontiguousarray(out_b.reshape(TK, 128).T).astype(np.float32),
        "cwih": _feat_major(np.asarray(inputs["cond_Wih"], np.float32)),
        "cwhh": _feat_major(np.asarray(inputs["cond_Whh"], np.float32)),
        "dwn": _feat_major(np.asarray(inputs["dec_Wih"][:, H:], np.float32)),
        "dwhh": _feat_major(np.asarray(inputs["dec_Whh"], np.float32)),
        "owt": _feat_major(np.asarray(inputs["out_W"], np.float32)),
    }
    dwe_fm = _feat_major(np.asarray(inputs["dec_Wih"][:, :H], np.float32))
    # slab m: [128, HK*128] so each DMA is one contiguous 256KB read
    shared["dwe"] = np.ascontiguousarray(
        dwe_fm.reshape(128, HK, G, 128).transpose(2, 0, 1, 3).reshape(
            G, 128, HK * 128))

    z_lv = z[:, np.arange(L) * L, 0, :]           # [B, L, Z]
    in_maps = []
    for c in range(NCORES):
        bs = slice(c * Bc, (c + 1) * Bc)
        zc = z_lv[bs]                              # [Bc, L, Z]
        zT = np.ascontiguousarray(
            zc.reshape(Bc, L, ZK, 128).transpose(3, 2, 1, 0).reshape(128, ZK, R)
        ).astype(bf16)
        h0 = dec_h0[:, bs, :]                      # [L, Bc, H]
        h0T = np.ascontiguousarray(
            h0.reshape(L, Bc, HK, 128).transpose(3, 2, 0, 1).reshape(128, HK, R))
        c0 = dec_c0[:, bs, :]
        c0T = np.ascontiguousarray(
            c0.reshape(L, Bc, HK, 128).transpose(3, 2, 0, 1).reshape(128, HK, R))
        m = dict(shared)
        m["zT"] = zT
        m["h0T"] = h0T.astype(bf16)
        m["c0T"] = c0T.astype(np.float32)
        in_maps.append(m)
    return in_maps


def _unpack_outputs(core_outs):
    notes = np.empty((B, L * NS, T), np.float32)
    for c, arr in enumerate(core_outs):
        # arr [NS, TK, 128, R] -> [Bc, L, NS, T]
        a = arr.astype(np.float32).reshape(NS, TK, 128, L, Bc).transpose(4, 3, 0, 1, 2)
        notes[c * Bc:(c + 1) * Bc] = a.reshape(Bc, L, NS, T).reshape(
            Bc, L * NS, T)
    return notes


def kernel(**inputs):
    nc = _build()
    in_maps = _pack_inputs(inputs)
    res = run_bass_kernel_spmd(nc, in_maps, list(range(NCORES)))
    return _unpack_outputs([r["outbuf"] for r in res.results])



# revision 4
# speedup vs baseline: 1.7230x; 1.7230x over previous
"""Trainium2 Bass kernel for nn_Decoder (MusicVAE-style hierarchical decoder).

Strategy (8 NeuronCores, data-parallel over batch, no inter-core comms):
  - Conductor LSTM (16 sequential levels, batch 32/core) computes per-level
    embeddings.
  - Decoder levels are INDEPENDENT (initial state from dec_h0/dec_c0,
    note0=0), so all 16 levels are batched: effective decoder batch
    16*32 = 512 rows per core, 16 sequential note steps.
  - The conductor embedding is constant within a level, so its gate
    contribution ge = emb @ dec_Wih[:, :H].T + dec_b is precomputed once.
  - Everything feature-major: [features on partitions, rows free].
  - All matmuls run in fp8 e4m3 with DoubleRow perf mode (2x PE throughput,
    K=256 per instruction).  Weights are pre-scaled by WS=32 on the host to
    avoid the e4m3 denormal zone; the 1/WS dequant is folded into the
    scalar_tensor_tensor gate adds / activation scale (exact powers of 2).
  - PSUM gate layout [128, 4, R] with slot order (i, f, o, g) so one fused
    DVE scalar_tensor_tensor does psum/WS + ge, and one fused sigmoid covers
    i,f,o.  Elementwise work is split across DVE / Scalar / GPSIMD.
  - c state in bf16; h/note/emb in fp8 (matmul operands), output in bf16.
"""
import numpy as np
import ml_dtypes

import concourse.bacc as bacc
import concourse.tile as tile
import concourse.mybir as mybir
from concourse.bass_utils import run_bass_kernel_spmd

bf16 = ml_dtypes.bfloat16
f8 = ml_dtypes.float8_e4m3
F32 = mybir.dt.float32
BF = mybir.dt.bfloat16
F8 = mybir.dt.float8e4
AF = mybir.ActivationFunctionType
ALU = mybir.AluOpType
DR = mybir.MatmulPerfMode.DoubleRow

NCORES = 8
B, Z, H, T = 256, 512, 1024, 512
L, NS = 16, 16
Bc = B // NCORES            # 32 batch rows per core
R = L * Bc                  # 512 decoder rows per core (levels x batch)
HK, TK, ZK = H // 128, T // 128, Z // 128   # 8, 4, 4
G = 4 * H // 128            # 32 gate chunks of 128
HP, TP, ZP = HK // 2, TK // 2, ZK // 2      # k-pair counts (fp8 DoubleRow)
WS = 32.0                   # fp8 weight pre-scale
IVS = 1.0 / WS

# gate slot order per p: (i, f, o, g) -> column chunk in 4H
def _gate_ms(p):
    return (p, HK + p, 3 * HK + p, 2 * HK + p)


# slot index for gate-group g (0:i 1:f 2:g 3:o) in the (i,f,o,g) psum layout
_SLOT_OF_GROUP = {0: 0, 1: 1, 2: 3, 3: 2}


def _declare(nc):
    d = {}
    ei = dict(kind="ExternalInput")
    d["ones"] = nc.dram_tensor("ones", [1, R], BF, **ei)
    d["cbias"] = nc.dram_tensor("cbias", [128, G], F32, **ei)
    d["dbias"] = nc.dram_tensor("dbias", [128, G], F32, **ei)
    d["obias"] = nc.dram_tensor("obias", [1, T], BF, **ei)     # 32*out_b
    d["zT"] = nc.dram_tensor("zT", [128, ZK, R], F8, **ei)
    d["h0T"] = nc.dram_tensor("h0T", [128, HK, R], F8, **ei)
    d["c0T"] = nc.dram_tensor("c0T", [128, HK, R], BF, **ei)
    d["cwih"] = nc.dram_tensor("cwih", [128, ZK, 4 * H], F8, **ei)
    d["cwhh"] = nc.dram_tensor("cwhh", [128, HK, 4 * H], F8, **ei)
    d["dwe"] = nc.dram_tensor("dwe", [G, 128, HK * 128], F8, **ei)
    d["dwn"] = nc.dram_tensor("dwn", [128, TK, 4 * H], F8, **ei)
    d["dwhh"] = nc.dram_tensor("dwhh", [128, HK, 4 * H], F8, **ei)
    d["owt"] = nc.dram_tensor("owt", [128, HK, T], F8, **ei)
    d["outbuf"] = nc.dram_tensor("outbuf", [NS, TK, 128, R], BF,
                                 kind="ExternalOutput")
    return d


def _mm8(nc, out, w, x, start, stop):
    return nc.tensor.matmul(out, w, x, start=start, stop=stop, perf_mode=DR)


def _body(nc, tc, d):
    import contextlib
    with contextlib.ExitStack() as ctx:
        Pp = ctx.enter_context(tc.tile_pool(name="persist", bufs=1))

        t_ones = Pp.tile([1, R], BF, tag="ones")
        nc.sync.dma_start(t_ones[:], d["ones"][:])
        t_ob = Pp.tile([1, T], BF, tag="obias")
        nc.sync.dma_start(t_ob[:], d["obias"][:])
        t_emb8 = Pp.tile([128, HK, R], F8, tag="emb8")
        t_h = [Pp.tile([128, HK, R], F8, tag=f"hT{i}", name=f"hT{i}")
               for i in (0, 1)]
        t_c = Pp.tile([128, HK, R], BF, tag="c")
        t_note8 = Pp.tile([128, TK, R], F8, tag="note8")
        t_noteb = Pp.tile([128, TK, R], BF, tag="noteb")
        nc.gpsimd.dma_start(t_h[0][:], d["h0T"][:])
        nc.gpsimd.dma_start(t_c[:], d["c0T"][:])

        # ---------------- conductor ----------------
        with tc.tile_pool(name="cond", bufs=1) as Pc, \
             tc.tile_pool(name="ctmp", bufs=3) as Pt, \
             tc.tile_pool(name="cps", bufs=4, space="PSUM") as PSc, \
             tc.tile_pool(name="gzps", bufs=4, space="PSUM") as PSz:
            t_cb = Pc.tile([128, G], F32, tag="cbias")
            nc.sync.dma_start(t_cb[:], d["cbias"][:])
            t_cwih = Pc.tile([128, ZK, 4 * H], F8, tag="cwih")
            nc.sync.dma_start(t_cwih[:], d["cwih"][:])
            t_zT = Pc.tile([128, ZK, R], F8, tag="zT")
            nc.sync.dma_start(t_zT[:], d["zT"][:])
            t_cwhh = Pc.tile([128, HK, 4 * H], F8, tag="cwhh")
            nc.sync.dma_start(t_cwhh[:], d["cwhh"][:])
            # gz laid out p-major with slots (i,f,o,g), like the decoder
            t_gz = Pc.tile([128, HK, 4, R], BF, tag="gz")
            t_cc = Pc.tile([128, HK, Bc], F32, tag="cc")

            # gz = z @ cond_Wih.T + cond_b for all levels at once (fp8 DR)
            for m in range(G):
                ms = slice(m * 128, (m + 1) * 128)
                ps = PSz.tile([128, R], F32, tag="gzp")
                for j in range(ZP):
                    _mm8(nc, ps[:], t_cwih[:, 2 * j:2 * j + 2, ms],
                         t_zT[:, 2 * j:2 * j + 2, :], j == 0, j == ZP - 1)
                p_, s_ = m % HK, _SLOT_OF_GROUP[m // HK]
                if m % 2 == 0:
                    nc.vector.tensor_scalar(t_gz[:, p_, s_, :], ps[:], IVS,
                                            t_cb[:, m:m + 1],
                                            op0=ALU.mult, op1=ALU.add)
                else:
                    nc.scalar.activation(t_gz[:, p_, s_, :], ps[:],
                                         AF.Identity, bias=t_cb[:, m:m + 1],
                                         scale=IVS)

            # sequential levels
            for lv in range(L):
                cs = slice(lv * Bc, (lv + 1) * Bc)
                ps_prev = slice((lv - 1) * Bc, lv * Bc)
                for p in range(HK):
                    gms = _gate_ms(p)
                    if lv == 0:
                        # h0 == 0: gates are just gz; c0 == 0: c = sig(i)*tanh(g)
                        sio = Pt.tile([128, 3, Bc], BF, tag="csio")
                        tg = Pt.tile([128, Bc], BF, tag="ctg")
                        nc.scalar.activation(sio[:], t_gz[:, p, 0:3, cs],
                                             AF.Sigmoid)
                        nc.scalar.activation(tg[:], t_gz[:, p, 3, cs], AF.Tanh)
                        nc.vector.tensor_mul(t_cc[:, p, :], sio[:, 0, :], tg[:])
                    else:
                        ps = PSc.tile([128, 4, Bc], F32, tag="cgp")
                        for si in range(4):
                            ms = slice(gms[si] * 128, (gms[si] + 1) * 128)
                            for j in range(HP):
                                _mm8(nc, ps[:, si, :],
                                     t_cwhh[:, 2 * j:2 * j + 2, ms],
                                     t_emb8[:, 2 * j:2 * j + 2, ps_prev],
                                     j == 0, j == HP - 1)
                        gs = Pt.tile([128, 4, Bc], BF, tag="cgs")
                        nc.vector.scalar_tensor_tensor(
                            gs[:], ps[:], IVS, t_gz[:, p, :, cs],
                            op0=ALU.mult, op1=ALU.add)
                        sio = Pt.tile([128, 3, Bc], BF, tag="csio")
                        tg = Pt.tile([128, Bc], BF, tag="ctg")
                        nc.scalar.activation(sio[:], gs[:, 0:3, :], AF.Sigmoid)
                        nc.scalar.activation(tg[:], gs[:, 3, :], AF.Tanh)
                        tm1 = Pt.tile([128, Bc], BF, tag="ctm1")
                        tm2 = Pt.tile([128, Bc], F32, tag="ctm2")
                        nc.vector.tensor_mul(tm1[:], sio[:, 0, :], tg[:])
                        nc.vector.tensor_mul(tm2[:], sio[:, 1, :], t_cc[:, p, :])
                        nc.vector.tensor_add(t_cc[:, p, :], tm1[:], tm2[:])
                    tcn = Pt.tile([128, Bc], BF, tag="ctcn")
                    nc.scalar.activation(tcn[:], t_cc[:, p, :], AF.Tanh)
                    nc.vector.tensor_mul(t_emb8[:, p, cs], sio[:, 2, :], tcn[:])

        # ge persists through the decoder (allocated after conductor frees)
        Pge = ctx.enter_context(tc.tile_pool(name="gepool", bufs=1))
        t_ge = Pge.tile([128, HK, 4, R], BF, tag="ge")

        # decoder weights (bulk, SWDGE queues; overlap with ge phase)
        Pw = ctx.enter_context(tc.tile_pool(name="wdec", bufs=1))
        t_dwn = Pw.tile([128, TK, 4 * H], F8, tag="dwn")
        nc.gpsimd.dma_start(t_dwn[:], d["dwn"][:])
        t_dwhh = Pw.tile([128, HK, 4 * H], F8, tag="dwhh")
        nc.gpsimd.dma_start(t_dwhh[:], d["dwhh"][:])
        t_owt = Pw.tile([128, HK, T], F8, tag="owt")
        nc.gpsimd.dma_start(t_owt[:], d["owt"][:])

        # ---------------- ge = emb @ dec_Wih[:, :H].T + dec_b ----------------
        with tc.tile_pool(name="gew", bufs=4) as Pgw, \
             tc.tile_pool(name="geps", bufs=4, space="PSUM") as PSg:
            t_db = Pgw.tile([128, G], F32, tag="dbias", bufs=1)
            nc.sync.dma_start(t_db[:], d["dbias"][:])
            for m in range(G):
                wt = Pgw.tile([128, HK, 128], F8, tag="dwe")
                nc.sync.dma_start(wt[:], d["dwe"][m].rearrange(
                    "p (k j) -> p k j", k=HK))
                ps = PSg.tile([128, R], F32, tag="gep")
                for j in range(HP):
                    _mm8(nc, ps[:], wt[:, 2 * j:2 * j + 2, :],
                         t_emb8[:, 2 * j:2 * j + 2, :], j == 0, j == HP - 1)
                p_, s_ = m % HK, _SLOT_OF_GROUP[m // HK]
                if m % 2 == 0:
                    nc.vector.tensor_scalar(t_ge[:, p_, s_, :], ps[:], IVS,
                                            t_db[:, m:m + 1],
                                            op0=ALU.mult, op1=ALU.add)
                else:
                    nc.scalar.activation(t_ge[:, p_, s_, :], ps[:],
                                         AF.Identity, bias=t_db[:, m:m + 1],
                                         scale=IVS)

        # ---------------- decoder: 16 note steps over 512 rows --------------
        with tc.tile_pool(name="dtmp", bufs=2) as Pdt, \
             tc.tile_pool(name="dps", bufs=2, space="PSUM") as PSd:
            for t in range(NS):
                hin = t_h[t % 2]
                hout = t_h[(t + 1) % 2]
                for p in range(HK):
                    gms = _gate_ms(p)
                    pt = PSd.tile([128, 4, R], F32, tag="dgp")
                    for si in range(4):
                        ms = slice(gms[si] * 128, (gms[si] + 1) * 128)
                        for j in range(HP):
                            _mm8(nc, pt[:, si, :],
                                 t_dwhh[:, 2 * j:2 * j + 2, ms],
                                 hin[:, 2 * j:2 * j + 2, :],
                                 j == 0, (j == HP - 1 and t == 0))
                        if t > 0:
                            for j in range(TP):
                                _mm8(nc, pt[:, si, :],
                                     t_dwn[:, 2 * j:2 * j + 2, ms],
                                     t_note8[:, 2 * j:2 * j + 2, :],
                                     False, j == TP - 1)
                    gs = Pdt.tile([128, 4, R], BF, tag="gs")
                    nc.vector.scalar_tensor_tensor(
                        gs[:], pt[:], IVS, t_ge[:, p, :, :],
                        op0=ALU.mult, op1=ALU.add)
                    sio = Pdt.tile([128, 3, R], BF, tag="sio")
                    tg = Pdt.tile([128, R], BF, tag="tg")
                    nc.scalar.activation(sio[:], gs[:, 0:3, :], AF.Sigmoid)
                    nc.scalar.activation(tg[:], gs[:, 3, :], AF.Tanh)
                    tm1 = Pdt.tile([128, R], BF, tag="tm1")
                    tm2 = Pdt.tile([128, R], BF, tag="tm2")
                    tcn = Pdt.tile([128, R], BF, tag="tcn")
                    nc.gpsimd.tensor_mul(tm1[:], sio[:, 0, :], tg[:])
                    nc.vector.tensor_mul(tm2[:], sio[:, 1, :], t_c[:, p, :])
                    nc.vector.tensor_add(t_c[:, p, :], tm1[:], tm2[:])
                    nc.scalar.activation(tcn[:], t_c[:, p, :], AF.Tanh)
                    nc.gpsimd.tensor_mul(hout[:, p, :], sio[:, 2, :], tcn[:])
                # output projection + sigmoid -> note (bf16 out, fp8 feedback)
                po = PSd.tile([128, TK, R], F32, tag="dgp", name="po")
                for tk in range(TK):
                    ts_ = slice(tk * 128, (tk + 1) * 128)
                    nc.tensor.matmul(po[:, tk, :], t_ob[0:1, ts_], t_ones[:],
                                     start=True, stop=False)
                    for j in range(HP):
                        _mm8(nc, po[:, tk, :], t_owt[:, 2 * j:2 * j + 2, ts_],
                             hout[:, 2 * j:2 * j + 2, :], False, j == HP - 1)
                nc.scalar.activation(t_noteb[:], po[:], AF.Sigmoid, scale=IVS)
                if t < NS - 1:
                    nc.gpsimd.tensor_copy(t_note8[:], t_noteb[:])
                for tk in range(TK):
                    nc.sync.dma_start(d["outbuf"][t, tk], t_noteb[:, tk, :])


_CACHE = {}


def _build():
    if "nc" not in _CACHE:
        nc = bacc.Bacc("TRN2", target_bir_lowering=False, debug=False,
                       num_devices=NCORES)
        d = _declare(nc)
        with tile.TileContext(nc) as tc:
            _body(nc, tc, d)
        nc.compile()
        _CACHE["nc"] = nc
    return _CACHE["nc"]


def _q8(x):
    return np.clip(x, -240.0, 240.0).astype(f8)


def _feat_major(W):
    """[J, K] -> [128, K/128, J] (stationary lhsT chunk layout)."""
    J, K = W.shape
    return np.ascontiguousarray(
        W.reshape(J, K // 128, 128).transpose(2, 1, 0))


def _pack_inputs(inputs):
    z = np.asarray(inputs["z"], np.float32)
    dec_h0 = np.asarray(inputs["dec_h0"], np.float32)
    dec_c0 = np.asarray(inputs["dec_c0"], np.float32)
    cond_b = np.asarray(inputs["cond_bih"] + inputs["cond_bhh"], np.float32)
    dec_b = np.asarray(inputs["dec_bih"] + inputs["dec_bhh"], np.float32)
    out_b = np.asarray(inputs["out_b"], np.float32)

    shared = {
        "ones": np.ones((1, R), dtype=bf16),
        "cbias": np.ascontiguousarray(cond_b.reshape(G, 128).T).astype(np.float32),
        "dbias": np.ascontiguousarray(dec_b.reshape(G, 128).T).astype(np.float32),
        "obias": (WS * out_b)[None, :].astype(bf16),
        "cwih": _q8(WS * _feat_major(np.asarray(inputs["cond_Wih"], np.float32))),
        "cwhh": _q8(WS * _feat_major(np.asarray(inputs["cond_Whh"], np.float32))),
        "dwn": _q8(WS * _feat_major(np.asarray(inputs["dec_Wih"][:, H:], np.float32))),
        "dwhh": _q8(WS * _feat_major(np.asarray(inputs["dec_Whh"], np.float32))),
        "owt": _q8(WS * _feat_major(np.asarray(inputs["out_W"], np.float32))),
    }
    dwe_fm = _q8(WS * _feat_major(np.asarray(inputs["dec_Wih"][:, :H], np.float32)))
    # slab m: [128, HK*128] so each DMA is one contiguous read
    shared["dwe"] = np.ascontiguousarray(
        dwe_fm.reshape(128, HK, G, 128).transpose(2, 0, 1, 3).reshape(
            G, 128, HK * 128))

    z_lv = z[:, np.arange(L) * L, 0, :]           # [B, L, Z]
    in_maps = []
    for c in range(NCORES):
        bs = slice(c * Bc, (c + 1) * Bc)
        zc = z_lv[bs]                              # [Bc, L, Z]
        zT = _q8(np.ascontiguousarray(
            zc.reshape(Bc, L, ZK, 128).transpose(3, 2, 1, 0).reshape(128, ZK, R)))
        h0 = dec_h0[:, bs, :]                      # [L, Bc, H]
        h0T = np.ascontiguousarray(
            h0.reshape(L, Bc, HK, 128).transpose(3, 2, 0, 1).reshape(128, HK, R))
        c0 = dec_c0[:, bs, :]
        c0T = np.ascontiguousarray(
            c0.reshape(L, Bc, HK, 128).transpose(3, 2, 0, 1).reshape(128, HK, R))
        m = dict(shared)
        m["zT"] = zT
        m["h0T"] = _q8(h0T)
        m["c0T"] = c0T.astype(bf16)
        in_maps.append(m)
    return in_maps


def _unpack_outputs(core_outs):
    notes = np.empty((B, L * NS, T), np.float32)
    for c, arr in enumerate(core_outs):
        # arr [NS, TK, 128, R] -> [Bc, L, NS, T]
        a = arr.astype(np.float32).reshape(NS, TK, 128, L, Bc).transpose(4, 3, 0, 1, 2)
        notes[c * Bc:(c + 1) * Bc] = a.reshape(Bc, L, NS, T).reshape(
            Bc, L * NS, T)
    return notes


def kernel(**inputs):
    nc = _build()
    in_maps = _pack_inputs(inputs)
    res = run_bass_kernel_spmd(nc, in_maps, list(range(NCORES)))
    return _unpack_outputs([r["outbuf"] for r in res.results])


# revision 5
# speedup vs baseline: 1.9321x; 1.1214x over previous
"""Trainium2 Bass kernel for nn_Decoder (MusicVAE-style hierarchical decoder).

Strategy (8 NeuronCores, data-parallel over batch, no inter-core comms):
  - Conductor LSTM (16 sequential levels, batch 32/core) computes per-level
    embeddings.
  - Decoder levels are INDEPENDENT (initial state from dec_h0/dec_c0,
    note0=0), so all 16 levels are batched: effective decoder batch
    16*32 = 512 rows per core, 16 sequential note steps.
  - The conductor embedding is constant within a level, so its gate
    contribution ge = emb @ dec_Wih[:, :H].T + dec_b is precomputed once.
  - Everything feature-major: [features on partitions, rows free].
  - All matmuls run in fp8 e4m3 with DoubleRow perf mode (2x PE throughput,
    K=256 per instruction).  Weights are pre-scaled by WS=32 on the host to
    avoid the e4m3 denormal zone; the 1/WS dequant is folded into the
    scalar_tensor_tensor gate adds / activation scale (exact powers of 2).
  - PSUM gate layout [128, 4, R] with slot order (i, f, o, g) so one fused
    DVE scalar_tensor_tensor does psum/WS + ge, and one fused sigmoid covers
    i,f,o.  Elementwise work is split across DVE / Scalar / GPSIMD.
  - c state in bf16; h/note/emb in fp8 (matmul operands), output in bf16.
"""
import numpy as np
import ml_dtypes

import concourse.bacc as bacc
import concourse.tile as tile
import concourse.mybir as mybir
from concourse.bass_utils import run_bass_kernel_spmd

bf16 = ml_dtypes.bfloat16
f8 = ml_dtypes.float8_e4m3
F32 = mybir.dt.float32
BF = mybir.dt.bfloat16
F8 = mybir.dt.float8e4
AF = mybir.ActivationFunctionType
ALU = mybir.AluOpType
DR = mybir.MatmulPerfMode.DoubleRow

NCORES = 8
B, Z, H, T = 256, 512, 1024, 512
L, NS = 16, 16
Bc = B // NCORES            # 32 batch rows per core
R = L * Bc                  # 512 decoder rows per core (levels x batch)
HK, TK, ZK = H // 128, T // 128, Z // 128   # 8, 4, 4
G = 4 * H // 128            # 32 gate chunks of 128
HP, TP, ZP = HK // 2, TK // 2, ZK // 2      # k-pair counts (fp8 DoubleRow)
WS = 32.0                   # fp8 weight pre-scale
IVS = 1.0 / WS

# gate slot order per p: (i, f, o, g) -> column chunk in 4H
def _gate_ms(p):
    return (p, HK + p, 3 * HK + p, 2 * HK + p)


# slot index for gate-group g (0:i 1:f 2:g 3:o) in the (i,f,o,g) psum layout
_SLOT_OF_GROUP = {0: 0, 1: 1, 2: 3, 3: 2}


def _declare(nc):
    d = {}
    ei = dict(kind="ExternalInput")
    d["ones"] = nc.dram_tensor("ones", [1, R], BF, **ei)
    d["cbias"] = nc.dram_tensor("cbias", [128, G], F32, **ei)
    d["dbias"] = nc.dram_tensor("dbias", [128, G], F32, **ei)
    d["obias"] = nc.dram_tensor("obias", [1, T], BF, **ei)     # 32*out_b
    d["zT"] = nc.dram_tensor("zT", [128, ZK, R], F8, **ei)
    d["h0T"] = nc.dram_tensor("h0T", [128, HK, R], F8, **ei)
    d["c0T"] = nc.dram_tensor("c0T", [128, HK, R], BF, **ei)
    d["cwih"] = nc.dram_tensor("cwih", [128, ZK, 4 * H], F8, **ei)
    d["cwhh"] = nc.dram_tensor("cwhh", [128, HK, 4 * H], F8, **ei)
    d["dwe"] = nc.dram_tensor("dwe", [G, 128, HK * 128], F8, **ei)
    d["dwn"] = nc.dram_tensor("dwn", [128, TK, 4 * H], F8, **ei)
    d["dwhh"] = nc.dram_tensor("dwhh", [128, HK, 4 * H], F8, **ei)
    d["owt"] = nc.dram_tensor("owt", [128, HK, T], F8, **ei)
    d["outbuf"] = nc.dram_tensor("outbuf", [NS, TK, 128, R], BF,
                                 kind="ExternalOutput")
    return d


def _mm8(nc, out, w, x, start, stop):
    return nc.tensor.matmul(out, w, x, start=start, stop=stop, perf_mode=DR)


def _body(nc, tc, d):
    import contextlib
    with contextlib.ExitStack() as ctx:
        Pp = ctx.enter_context(tc.tile_pool(name="persist", bufs=1))

        t_ones = Pp.tile([1, R], BF, tag="ones")
        nc.sync.dma_start(t_ones[:], d["ones"][:])
        t_ob = Pp.tile([1, T], BF, tag="obias")
        nc.sync.dma_start(t_ob[:], d["obias"][:])
        t_emb8 = Pp.tile([128, HK, R], F8, tag="emb8")
        t_h = [Pp.tile([128, HK, R], F8, tag=f"hT{i}", name=f"hT{i}")
               for i in (0, 1)]
        t_c = Pp.tile([128, HK, R], BF, tag="c")
        t_note8 = Pp.tile([128, TK, R], F8, tag="note8")
        t_noteb = Pp.tile([128, TK, R], BF, tag="noteb")
        nc.gpsimd.dma_start(t_h[0][:], d["h0T"][:])
        nc.gpsimd.dma_start(t_c[:], d["c0T"][:])

        # ---------------- conductor ----------------
        with tc.tile_pool(name="cond", bufs=1) as Pc, \
             tc.tile_pool(name="ctmp", bufs=3) as Pt, \
             tc.tile_pool(name="cps", bufs=4, space="PSUM") as PSc, \
             tc.tile_pool(name="gzps", bufs=4, space="PSUM") as PSz:
            t_cb = Pc.tile([128, G], F32, tag="cbias")
            nc.sync.dma_start(t_cb[:], d["cbias"][:])
            t_cwih = Pc.tile([128, ZK, 4 * H], F8, tag="cwih")
            nc.sync.dma_start(t_cwih[:], d["cwih"][:])
            t_zT = Pc.tile([128, ZK, R], F8, tag="zT")
            nc.sync.dma_start(t_zT[:], d["zT"][:])
            t_cwhh = Pc.tile([128, HK, 4 * H], F8, tag="cwhh")
            nc.sync.dma_start(t_cwhh[:], d["cwhh"][:])
            # gz laid out p-major with slots (i,f,o,g), like the decoder
            t_gz = Pc.tile([128, HK, 4, R], BF, tag="gz")
            t_cc = Pc.tile([128, HK, Bc], F32, tag="cc")

            # gz = z @ cond_Wih.T + cond_b for all levels at once (fp8 DR)
            for m in range(G):
                ms = slice(m * 128, (m + 1) * 128)
                ps = PSz.tile([128, R], F32, tag="gzp")
                for j in range(ZP):
                    _mm8(nc, ps[:], t_cwih[:, 2 * j:2 * j + 2, ms],
                         t_zT[:, 2 * j:2 * j + 2, :], j == 0, j == ZP - 1)
                p_, s_ = m % HK, _SLOT_OF_GROUP[m // HK]
                if m % 2 == 0:
                    nc.vector.tensor_scalar(t_gz[:, p_, s_, :], ps[:], IVS,
                                            t_cb[:, m:m + 1],
                                            op0=ALU.mult, op1=ALU.add)
                else:
                    nc.scalar.activation(t_gz[:, p_, s_, :], ps[:],
                                         AF.Identity, bias=t_cb[:, m:m + 1],
                                         scale=IVS)

            # sequential levels
            for lv in range(L):
                cs = slice(lv * Bc, (lv + 1) * Bc)
                ps_prev = slice((lv - 1) * Bc, lv * Bc)
                for p in range(HK):
                    gms = _gate_ms(p)
                    if lv == 0:
                        # h0 == 0: gates are just gz; c0 == 0: c = sig(i)*tanh(g)
                        sio = Pt.tile([128, 3, Bc], BF, tag="csio")
                        tg = Pt.tile([128, Bc], BF, tag="ctg")
                        nc.scalar.activation(sio[:], t_gz[:, p, 0:3, cs],
                                             AF.Sigmoid)
                        nc.scalar.activation(tg[:], t_gz[:, p, 3, cs], AF.Tanh)
                        nc.vector.tensor_mul(t_cc[:, p, :], sio[:, 0, :], tg[:])
                    else:
                        ps = PSc.tile([128, 4, Bc], F32, tag="cgp")
                        for si in range(4):
                            ms = slice(gms[si] * 128, (gms[si] + 1) * 128)
                            for j in range(HP):
                                _mm8(nc, ps[:, si, :],
                                     t_cwhh[:, 2 * j:2 * j + 2, ms],
                                     t_emb8[:, 2 * j:2 * j + 2, ps_prev],
                                     j == 0, j == HP - 1)
                        gs = Pt.tile([128, 4, Bc], BF, tag="cgs")
                        nc.vector.scalar_tensor_tensor(
                            gs[:], ps[:], IVS, t_gz[:, p, :, cs],
                            op0=ALU.mult, op1=ALU.add)
                        sio = Pt.tile([128, 3, Bc], BF, tag="csio")
                        tg = Pt.tile([128, Bc], BF, tag="ctg")
                        nc.scalar.activation(sio[:], gs[:, 0:3, :], AF.Sigmoid)
                        nc.scalar.activation(tg[:], gs[:, 3, :], AF.Tanh)
                        tm1 = Pt.tile([128, Bc], BF, tag="ctm1")
                        tm2 = Pt.tile([128, Bc], F32, tag="ctm2")
                        nc.vector.tensor_mul(tm1[:], sio[:, 0, :], tg[:])
                        nc.vector.tensor_mul(tm2[:], sio[:, 1, :], t_cc[:, p, :])
                        nc.vector.tensor_add(t_cc[:, p, :], tm1[:], tm2[:])
                    tcn = Pt.tile([128, Bc], BF, tag="ctcn")
                    nc.scalar.activation(tcn[:], t_cc[:, p, :], AF.Tanh)
                    nc.vector.tensor_mul(t_emb8[:, p, cs], sio[:, 2, :], tcn[:])

        # ge persists through the decoder (allocated after conductor frees)
        Pge = ctx.enter_context(tc.tile_pool(name="gepool", bufs=1))
        t_ge = Pge.tile([128, HK, 4, R], BF, tag="ge")

        # decoder weights (bulk, SWDGE queues; overlap with ge phase)
        Pw = ctx.enter_context(tc.tile_pool(name="wdec", bufs=1))
        t_dwn = Pw.tile([128, TK, 4 * H], F8, tag="dwn")
        nc.gpsimd.dma_start(t_dwn[:], d["dwn"][:])
        t_dwhh = Pw.tile([128, HK, 4 * H], F8, tag="dwhh")
        nc.gpsimd.dma_start(t_dwhh[:], d["dwhh"][:])
        t_owt = Pw.tile([128, HK, T], F8, tag="owt")
        nc.gpsimd.dma_start(t_owt[:], d["owt"][:])

        # ---------------- ge = emb @ dec_Wih[:, :H].T + dec_b ----------------
        with tc.tile_pool(name="gew", bufs=4) as Pgw, \
             tc.tile_pool(name="geps", bufs=4, space="PSUM") as PSg:
            t_db = Pgw.tile([128, G], F32, tag="dbias", bufs=1)
            nc.sync.dma_start(t_db[:], d["dbias"][:])
            for m in range(G):
                wt = Pgw.tile([128, HK, 128], F8, tag="dwe")
                nc.sync.dma_start(wt[:], d["dwe"][m].rearrange(
                    "p (k j) -> p k j", k=HK))
                ps = PSg.tile([128, R], F32, tag="gep")
                for j in range(HP):
                    _mm8(nc, ps[:], wt[:, 2 * j:2 * j + 2, :],
                         t_emb8[:, 2 * j:2 * j + 2, :], j == 0, j == HP - 1)
                p_, s_ = m % HK, _SLOT_OF_GROUP[m // HK]
                if m % 2 == 0:
                    nc.vector.tensor_scalar(t_ge[:, p_, s_, :], ps[:], IVS,
                                            t_db[:, m:m + 1],
                                            op0=ALU.mult, op1=ALU.add)
                else:
                    nc.scalar.activation(t_ge[:, p_, s_, :], ps[:],
                                         AF.Identity, bias=t_db[:, m:m + 1],
                                         scale=IVS)

        # ---------------- decoder: 16 note steps over 512 rows --------------
        # Software-pipelined emission: each engine's in-order queue never
        # waits mid-chain.  Stage A(p) = STT + activations issued right after
        # p's matmuls; stage B(p) = c-update/h-mul issued one p later.
        with tc.tile_pool(name="dtmp", bufs=3) as Pdt, \
             tc.tile_pool(name="dps", bufs=2, space="PSUM") as PSd:
            stage = {}

            def stage_a(t, p, pt):
                gs = Pdt.tile([128, 4, R], BF, tag="gs")
                nc.vector.scalar_tensor_tensor(
                    gs[:], pt[:], IVS, t_ge[:, p, :, :],
                    op0=ALU.mult, op1=ALU.add)
                sio = Pdt.tile([128, 3, R], BF, tag="sio")
                tg = Pdt.tile([128, R], BF, tag="tg")
                nc.scalar.activation(sio[:], gs[:, 0:3, :], AF.Sigmoid)
                nc.scalar.activation(tg[:], gs[:, 3, :], AF.Tanh)
                tm1 = Pdt.tile([128, R], BF, tag="tm1")
                nc.gpsimd.tensor_mul(tm1[:], sio[:, 0, :], tg[:])
                stage[p] = (sio, tm1)

            def stage_b(t, p, hout):
                sio, tm1 = stage.pop(p)
                tm2 = Pdt.tile([128, R], BF, tag="tm2")
                tcn = Pdt.tile([128, R], BF, tag="tcn")
                nc.vector.tensor_mul(tm2[:], sio[:, 1, :], t_c[:, p, :])
                nc.vector.tensor_add(t_c[:, p, :], tm1[:], tm2[:])
                nc.scalar.activation(tcn[:], t_c[:, p, :], AF.Tanh)
                nc.gpsimd.tensor_mul(hout[:, p, :], sio[:, 2, :], tcn[:])

            for t in range(NS):
                hin = t_h[t % 2]
                hout = t_h[(t + 1) % 2]
                for p in range(HK):
                    gms = _gate_ms(p)
                    pt = PSd.tile([128, 4, R], F32, tag="dgp")
                    for si in range(4):
                        ms = slice(gms[si] * 128, (gms[si] + 1) * 128)
                        if t > 0:
                            # note contribution first: note8(t-1) is the
                            # freshest input, h pairs follow
                            for j in range(TP):
                                _mm8(nc, pt[:, si, :],
                                     t_dwn[:, 2 * j:2 * j + 2, ms],
                                     t_note8[:, 2 * j:2 * j + 2, :],
                                     j == 0, False)
                        for j in range(HP):
                            _mm8(nc, pt[:, si, :],
                                 t_dwhh[:, 2 * j:2 * j + 2, ms],
                                 hin[:, 2 * j:2 * j + 2, :],
                                 (j == 0 and t == 0), j == HP - 1)
                    stage_a(t, p, pt)
                    if p >= 1:
                        stage_b(t, p - 1, hout)
                stage_b(t, HK - 1, hout)
                # output projection + sigmoid -> fp8 feedback + bf16 out
                po = PSd.tile([128, TK, R], F32, tag="dgp", name="po")
                for tk in range(TK):
                    ts_ = slice(tk * 128, (tk + 1) * 128)
                    nc.tensor.matmul(po[:, tk, :], t_ob[0:1, ts_], t_ones[:],
                                     start=True, stop=False)
                    for j in range(HP):
                        _mm8(nc, po[:, tk, :], t_owt[:, 2 * j:2 * j + 2, ts_],
                             hout[:, 2 * j:2 * j + 2, :], False, j == HP - 1)
                if t < NS - 1:
                    nc.scalar.activation(t_note8[:], po[:], AF.Sigmoid,
                                         scale=IVS)
                nc.scalar.activation(t_noteb[:], po[:], AF.Sigmoid, scale=IVS)
                for tk in range(TK):
                    nc.sync.dma_start(d["outbuf"][t, tk], t_noteb[:, tk, :])


_CACHE = {}


def _build():
    if "nc" not in _CACHE:
        nc = bacc.Bacc("TRN2", target_bir_lowering=False, debug=False,
                       num_devices=NCORES)
        d = _declare(nc)
        with tile.TileContext(nc) as tc:
            _body(nc, tc, d)
        nc.compile()
        _CACHE["nc"] = nc
    return _CACHE["nc"]


def _q8(x):
    return np.clip(x, -240.0, 240.0).astype(f8)


def _feat_major(W):
    """[J, K] -> [128, K/128, J] (stationary lhsT chunk layout)."""
    J, K = W.shape
    return np.ascontiguousarray(
        W.reshape(J, K // 128, 128).transpose(2, 1, 0))


def _pack_inputs(inputs):
    z = np.asarray(inputs["z"], np.float32)
    dec_h0 = np.asarray(inputs["dec_h0"], np.float32)
    dec_c0 = np.asarray(inputs["dec_c0"], np.float32)
    cond_b = np.asarray(inputs["cond_bih"] + inputs["cond_bhh"], np.float32)
    dec_b = np.asarray(inputs["dec_bih"] + inputs["dec_bhh"], np.float32)
    out_b = np.asarray(inputs["out_b"], np.float32)

    shared = {
        "ones": np.ones((1, R), dtype=bf16),
        "cbias": np.ascontiguousarray(cond_b.reshape(G, 128).T).astype(np.float32),
        "dbias": np.ascontiguousarray(dec_b.reshape(G, 128).T).astype(np.float32),
        "obias": (WS * out_b)[None, :].astype(bf16),
        "cwih": _q8(WS * _feat_major(np.asarray(inputs["cond_Wih"], np.float32))),
        "cwhh": _q8(WS * _feat_major(np.asarray(inputs["cond_Whh"], np.float32))),
        "dwn": _q8(WS * _feat_major(np.asarray(inputs["dec_Wih"][:, H:], np.float32))),
        "dwhh": _q8(WS * _feat_major(np.asarray(inputs["dec_Whh"], np.float32))),
        "owt": _q8(WS * _feat_major(np.asarray(inputs["out_W"], np.float32))),
    }
    dwe_fm = _q8(WS * _feat_major(np.asarray(inputs["dec_Wih"][:, :H], np.float32)))
    # slab m: [128, HK*128] so each DMA is one contiguous read
    shared["dwe"] = np.ascontiguousarray(
        dwe_fm.reshape(128, HK, G, 128).transpose(2, 0, 1, 3).reshape(
            G, 128, HK * 128))

    z_lv = z[:, np.arange(L) * L, 0, :]           # [B, L, Z]
    in_maps = []
    for c in range(NCORES):
        bs = slice(c * Bc, (c + 1) * Bc)
        zc = z_lv[bs]                              # [Bc, L, Z]
        zT = _q8(np.ascontiguousarray(
            zc.reshape(Bc, L, ZK, 128).transpose(3, 2, 1, 0).reshape(128, ZK, R)))
        h0 = dec_h0[:, bs, :]                      # [L, Bc, H]
        h0T = np.ascontiguousarray(
            h0.reshape(L, Bc, HK, 128).transpose(3, 2, 0, 1).reshape(128, HK, R))
        c0 = dec_c0[:, bs, :]
        c0T = np.ascontiguousarray(
            c0.reshape(L, Bc, HK, 128).transpose(3, 2, 0, 1).reshape(128, HK, R))
        m = dict(shared)
        m["zT"] = zT
        m["h0T"] = _q8(h0T)
        m["c0T"] = c0T.astype(bf16)
        in_maps.append(m)
    return in_maps


def _unpack_outputs(core_outs):
    notes = np.empty((B, L * NS, T), np.float32)
    for c, arr in enumerate(core_outs):
        # arr [NS, TK, 128, R] -> [Bc, L, NS, T]
        a = arr.astype(np.float32).reshape(NS, TK, 128, L, Bc).transpose(4, 3, 0, 1, 2)
        notes[c * Bc:(c + 1) * Bc] = a.reshape(Bc, L, NS, T).reshape(
            Bc, L * NS, T)
    return notes


def kernel(**inputs):
    nc = _build()
    in_maps = _pack_inputs(inputs)
    res = run_bass_kernel_spmd(nc, in_maps, list(range(NCORES)))
    return _unpack_outputs([r["outbuf"] for r in res.results])


# revision 6
# speedup vs baseline: 2.4186x; 1.2518x over previous
"""Trainium2 Bass kernel for nn_Decoder (MusicVAE-style hierarchical decoder).

Strategy (8 NeuronCores, data-parallel over batch, no inter-core comms):
  - Conductor LSTM (16 sequential levels, batch 32/core) computes per-level
    embeddings.
  - Decoder levels are INDEPENDENT (initial state from dec_h0/dec_c0,
    note0=0), so all 16 levels are batched: effective decoder batch
    16*32 = 512 rows per core, 16 sequential note steps.
  - The conductor embedding is constant within a level, so its gate
    contribution ge = emb @ dec_Wih[:, :H].T + dec_b is precomputed once.
  - Everything feature-major: [features on partitions, rows free].
  - All matmuls run in fp8 e4m3 with DoubleRow perf mode (2x PE throughput,
    K=256 per instruction).  Weights are pre-scaled by WS=32 on the host to
    avoid the e4m3 denormal zone; the 1/WS dequant is folded into the
    scalar_tensor_tensor gate adds / activation scale (exact powers of 2).
  - PSUM gate layout [128, 4, R] with slot order (i, f, o, g) so one fused
    DVE scalar_tensor_tensor does psum/WS + ge, and one fused sigmoid covers
    i,f,o.  Elementwise work is split across DVE / Scalar / GPSIMD.
  - c state in bf16; h/note/emb in fp8 (matmul operands), output in bf16.
"""
import numpy as np
import ml_dtypes

import concourse.bacc as bacc
import concourse.tile as tile
import concourse.mybir as mybir
from concourse.bass_utils import run_bass_kernel_spmd

bf16 = ml_dtypes.bfloat16
f8 = ml_dtypes.float8_e4m3
F32 = mybir.dt.float32
BF = mybir.dt.bfloat16
F8 = mybir.dt.float8e4
AF = mybir.ActivationFunctionType
ALU = mybir.AluOpType
DR = mybir.MatmulPerfMode.DoubleRow

NCORES = 8
B, Z, H, T = 256, 512, 1024, 512
L, NS = 16, 16
Bc = B // NCORES            # 32 batch rows per core
R = L * Bc                  # 512 decoder rows per core (levels x batch)
HK, TK, ZK = H // 128, T // 128, Z // 128   # 8, 4, 4
G = 4 * H // 128            # 32 gate chunks of 128
HP, TP, ZP = HK // 2, TK // 2, ZK // 2      # k-pair counts (fp8 DoubleRow)
WS = 32.0                   # fp8 weight pre-scale
IVS = 1.0 / WS

# gate slot order per p: (i, f, o, g) -> column chunk in 4H
def _gate_ms(p):
    return (p, HK + p, 3 * HK + p, 2 * HK + p)


# slot index for gate-group g (0:i 1:f 2:g 3:o) in the (i,f,o,g) psum layout
_SLOT_OF_GROUP = {0: 0, 1: 1, 2: 3, 3: 2}


def _declare(nc):
    d = {}
    ei = dict(kind="ExternalInput")
    d["ones"] = nc.dram_tensor("ones", [1, R], BF, **ei)
    d["cbias"] = nc.dram_tensor("cbias", [128, G], F32, **ei)
    d["dbias"] = nc.dram_tensor("dbias", [128, G], F32, **ei)
    d["obias"] = nc.dram_tensor("obias", [1, T], BF, **ei)     # 32*out_b
    d["zT"] = nc.dram_tensor("zT", [128, ZK, R], F8, **ei)
    d["h0T"] = nc.dram_tensor("h0T", [128, HK, R], F8, **ei)
    d["c0T"] = nc.dram_tensor("c0T", [128, HK, R], BF, **ei)
    d["cwih"] = nc.dram_tensor("cwih", [128, ZK, 4 * H], F8, **ei)
    d["cwhh"] = nc.dram_tensor("cwhh", [128, HK, 4 * H], F8, **ei)
    d["dwe"] = nc.dram_tensor("dwe", [G, 128, HK * 128], F8, **ei)
    d["dwn"] = nc.dram_tensor("dwn", [128, TK, 4 * H], F8, **ei)
    d["dwhh"] = nc.dram_tensor("dwhh", [128, HK, 4 * H], F8, **ei)
    d["owt"] = nc.dram_tensor("owt", [128, HK, T], F8, **ei)
    d["outbuf"] = nc.dram_tensor("outbuf", [NS, TK, 128, R], BF,
                                 kind="ExternalOutput")
    return d


def _mm8(nc, out, w, x, start, stop):
    return nc.tensor.matmul(out, w, x, start=start, stop=stop, perf_mode=DR)


def _body(nc, tc, d):
    import contextlib
    with contextlib.ExitStack() as ctx:
        Pp = ctx.enter_context(tc.tile_pool(name="persist", bufs=1))

        t_ones = Pp.tile([1, R], BF, tag="ones")
        nc.sync.dma_start(t_ones[:], d["ones"][:])
        t_ob = Pp.tile([1, T], BF, tag="obias")
        nc.sync.dma_start(t_ob[:], d["obias"][:])
        t_emb8 = Pp.tile([128, HK, R], F8, tag="emb8")
        t_h = [Pp.tile([128, HK, R], F8, tag=f"hT{i}", name=f"hT{i}")
               for i in (0, 1)]
        t_c = Pp.tile([128, HK, R], BF, tag="c")
        t_note8 = Pp.tile([128, TK, R], F8, tag="note8")
        t_noteb = Pp.tile([128, TK, R], BF, tag="noteb")
        nc.gpsimd.dma_start(t_h[0][:], d["h0T"][:])
        nc.gpsimd.dma_start(t_c[:], d["c0T"][:])

        # ---------------- conductor ----------------
        with tc.tile_pool(name="cond", bufs=1) as Pc, \
             tc.tile_pool(name="ctmp", bufs=3) as Pt, \
             tc.tile_pool(name="cps", bufs=4, space="PSUM") as PSc, \
             tc.tile_pool(name="gzps", bufs=4, space="PSUM") as PSz:
            t_cb = Pc.tile([128, G], F32, tag="cbias")
            nc.sync.dma_start(t_cb[:], d["cbias"][:])
            t_cwih = Pc.tile([128, ZK, 4 * H], F8, tag="cwih")
            nc.sync.dma_start(t_cwih[:], d["cwih"][:])
            t_zT = Pc.tile([128, ZK, R], F8, tag="zT")
            nc.sync.dma_start(t_zT[:], d["zT"][:])
            t_cwhh = Pc.tile([128, HK, 4 * H], F8, tag="cwhh")
            nc.sync.dma_start(t_cwhh[:], d["cwhh"][:])
            # gz laid out p-major with slots (i,f,o,g), like the decoder
            t_gz = Pc.tile([128, HK, 4, R], BF, tag="gz")
            t_cc = Pc.tile([128, HK, Bc], F32, tag="cc")

            # gz = z @ cond_Wih.T + cond_b for all levels at once (fp8 DR)
            for m in range(G):
                ms = slice(m * 128, (m + 1) * 128)
                ps = PSz.tile([128, R], F32, tag="gzp")
                for j in range(ZP):
                    _mm8(nc, ps[:], t_cwih[:, 2 * j:2 * j + 2, ms],
                         t_zT[:, 2 * j:2 * j + 2, :], j == 0, j == ZP - 1)
                p_, s_ = m % HK, _SLOT_OF_GROUP[m // HK]
                if m % 2 == 0:
                    nc.vector.tensor_scalar(t_gz[:, p_, s_, :], ps[:], IVS,
                                            t_cb[:, m:m + 1],
                                            op0=ALU.mult, op1=ALU.add)
                else:
                    nc.scalar.activation(t_gz[:, p_, s_, :], ps[:],
                                         AF.Identity, bias=t_cb[:, m:m + 1],
                                         scale=IVS)

            # sequential levels
            for lv in range(L):
                cs = slice(lv * Bc, (lv + 1) * Bc)
                ps_prev = slice((lv - 1) * Bc, lv * Bc)
                for p in range(HK):
                    gms = _gate_ms(p)
                    if lv == 0:
                        # h0 == 0: gates are just gz; c0 == 0: c = sig(i)*tanh(g)
                        sio = Pt.tile([128, 3, Bc], BF, tag="csio")
                        tg = Pt.tile([128, Bc], BF, tag="ctg")
                        nc.scalar.activation(sio[:], t_gz[:, p, 0:3, cs],
                                             AF.Sigmoid)
                        nc.scalar.activation(tg[:], t_gz[:, p, 3, cs], AF.Tanh)
                        nc.vector.tensor_mul(t_cc[:, p, :], sio[:, 0, :], tg[:])
                    else:
                        ps = PSc.tile([128, 4, Bc], F32, tag="cgp")
                        for si in range(4):
                            ms = slice(gms[si] * 128, (gms[si] + 1) * 128)
                            for j in range(HP):
                                _mm8(nc, ps[:, si, :],
                                     t_cwhh[:, 2 * j:2 * j + 2, ms],
                                     t_emb8[:, 2 * j:2 * j + 2, ps_prev],
                                     j == 0, j == HP - 1)
                        gs = Pt.tile([128, 4, Bc], BF, tag="cgs")
                        nc.vector.scalar_tensor_tensor(
                            gs[:], ps[:], IVS, t_gz[:, p, :, cs],
                            op0=ALU.mult, op1=ALU.add)
                        sio = Pt.tile([128, 3, Bc], BF, tag="csio")
                        tg = Pt.tile([128, Bc], BF, tag="ctg")
                        nc.scalar.activation(sio[:], gs[:, 0:3, :], AF.Sigmoid)
                        nc.scalar.activation(tg[:], gs[:, 3, :], AF.Tanh)
                        tm1 = Pt.tile([128, Bc], BF, tag="ctm1")
                        tm2 = Pt.tile([128, Bc], F32, tag="ctm2")
                        nc.vector.tensor_mul(tm1[:], sio[:, 0, :], tg[:])
                        nc.vector.tensor_mul(tm2[:], sio[:, 1, :], t_cc[:, p, :])
                        nc.vector.tensor_add(t_cc[:, p, :], tm1[:], tm2[:])
                    tcn = Pt.tile([128, Bc], BF, tag="ctcn")
                    nc.scalar.activation(tcn[:], t_cc[:, p, :], AF.Tanh)
                    nc.vector.tensor_mul(t_emb8[:, p, cs], sio[:, 2, :], tcn[:])

        # ge persists through the decoder (allocated after conductor frees)
        Pge = ctx.enter_context(tc.tile_pool(name="gepool", bufs=1))
        t_ge = Pge.tile([128, HK, 4, R], BF, tag="ge")

        # decoder weights (bulk, SWDGE queues; overlap with ge phase)
        Pw = ctx.enter_context(tc.tile_pool(name="wdec", bufs=1))
        t_dwn = Pw.tile([128, TK, 4 * H], F8, tag="dwn")
        nc.gpsimd.dma_start(t_dwn[:], d["dwn"][:])
        t_dwhh = Pw.tile([128, HK, 4 * H], F8, tag="dwhh")
        nc.gpsimd.dma_start(t_dwhh[:], d["dwhh"][:])
        t_owt = Pw.tile([128, HK, T], F8, tag="owt")
        nc.gpsimd.dma_start(t_owt[:], d["owt"][:])

        # ---------------- ge = emb @ dec_Wih[:, :H].T + dec_b ----------------
        with tc.tile_pool(name="gew", bufs=4) as Pgw, \
             tc.tile_pool(name="geps", bufs=4, space="PSUM") as PSg:
            t_db = Pgw.tile([128, G], F32, tag="dbias", bufs=1)
            nc.sync.dma_start(t_db[:], d["dbias"][:])
            for m in range(G):
                wt = Pgw.tile([128, HK, 128], F8, tag="dwe")
                nc.sync.dma_start(wt[:], d["dwe"][m].rearrange(
                    "p (k j) -> p k j", k=HK))
                ps = PSg.tile([128, R], F32, tag="gep")
                for j in range(HP):
                    _mm8(nc, ps[:], wt[:, 2 * j:2 * j + 2, :],
                         t_emb8[:, 2 * j:2 * j + 2, :], j == 0, j == HP - 1)
                p_, s_ = m % HK, _SLOT_OF_GROUP[m // HK]
                if m % 2 == 0:
                    nc.vector.tensor_scalar(t_ge[:, p_, s_, :], ps[:], IVS,
                                            t_db[:, m:m + 1],
                                            op0=ALU.mult, op1=ALU.add)
                else:
                    nc.scalar.activation(t_ge[:, p_, s_, :], ps[:],
                                         AF.Identity, bias=t_db[:, m:m + 1],
                                         scale=IVS)

        # ---------------- decoder: 16 note steps over 512 rows --------------
        # The 16 levels split into two independent row-cohorts (RC=256 rows
        # each) whose steps interleave: cohort A's serial drain/oproj tail
        # hides under cohort B's matmul phase.  Within a cohort-step the
        # emission is software-pipelined (stage A at p, stage B at p-1) so
        # each engine's in-order queue never waits mid-chain.
        RC = R // 2
        with tc.tile_pool(name="dtmp", bufs=3) as Pdt, \
             tc.tile_pool(name="dps", bufs=4, space="PSUM") as PSd:
            stage = {}

            def stage_a(rs, p, pt):
                gs = Pdt.tile([128, 4, RC], BF, tag="gs")
                nc.vector.scalar_tensor_tensor(
                    gs[:], pt[:], IVS, t_ge[:, p, :, rs],
                    op0=ALU.mult, op1=ALU.add)
                sio = Pdt.tile([128, 3, RC], BF, tag="sio")
                tg = Pdt.tile([128, RC], BF, tag="tg")
                nc.scalar.activation(sio[:], gs[:, 0:3, :], AF.Sigmoid)
                nc.scalar.activation(tg[:], gs[:, 3, :], AF.Tanh)
                tm1 = Pdt.tile([128, RC], BF, tag="tm1")
                nc.gpsimd.tensor_mul(tm1[:], sio[:, 0, :], tg[:])
                stage[p] = (sio, tm1)

            def stage_b(rs, p, hout):
                sio, tm1 = stage.pop(p)
                tm2 = Pdt.tile([128, RC], BF, tag="tm2")
                tcn = Pdt.tile([128, RC], BF, tag="tcn")
                nc.vector.tensor_mul(tm2[:], sio[:, 1, :], t_c[:, p, rs])
                nc.vector.tensor_add(t_c[:, p, rs], tm1[:], tm2[:])
                nc.scalar.activation(tcn[:], t_c[:, p, rs], AF.Tanh)
                nc.gpsimd.tensor_mul(hout[:, p, rs], sio[:, 2, :], tcn[:])

            def cohort_step(t, rh):
                rs = slice(rh * RC, (rh + 1) * RC)
                hin = t_h[t % 2]
                hout = t_h[(t + 1) % 2]
                for p in range(HK):
                    gms = _gate_ms(p)
                    pt = PSd.tile([128, 4, RC], F32, tag="dgp")
                    for si in range(4):
                        ms = slice(gms[si] * 128, (gms[si] + 1) * 128)
                        if t > 0:
                            # note contribution first: note8(t-1) is the
                            # freshest input, h pairs follow
                            for j in range(TP):
                                _mm8(nc, pt[:, si, :],
                                     t_dwn[:, 2 * j:2 * j + 2, ms],
                                     t_note8[:, 2 * j:2 * j + 2, rs],
                                     j == 0, False)
                        for j in range(HP):
                            _mm8(nc, pt[:, si, :],
                                 t_dwhh[:, 2 * j:2 * j + 2, ms],
                                 hin[:, 2 * j:2 * j + 2, rs],
                                 (j == 0 and t == 0), j == HP - 1)
                    stage_a(rs, p, pt)
                    if p >= 1:
                        stage_b(rs, p - 1, hout)
                stage_b(rs, HK - 1, hout)
                # output projection + sigmoid -> fp8 feedback + bf16 out
                po = PSd.tile([128, TK, RC], F32, tag="dgp", name="po")
                for tk in range(TK):
                    ts_ = slice(tk * 128, (tk + 1) * 128)
                    nc.tensor.matmul(po[:, tk, :], t_ob[0:1, ts_],
                                     t_ones[0:1, rs], start=True, stop=False)
                    for j in range(HP):
                        _mm8(nc, po[:, tk, :], t_owt[:, 2 * j:2 * j + 2, ts_],
                             hout[:, 2 * j:2 * j + 2, rs], False, j == HP - 1)
                if t < NS - 1:
                    nc.scalar.activation(t_note8[:, :, rs], po[:], AF.Sigmoid,
                                         scale=IVS)
                nc.scalar.activation(t_noteb[:, :, rs], po[:], AF.Sigmoid,
                                     scale=IVS)
                for tk in range(TK):
                    nc.sync.dma_start(d["outbuf"][t, tk, :, rs],
                                      t_noteb[:, tk, rs])

            for t in range(NS):
                cohort_step(t, 0)
                cohort_step(t, 1)


_CACHE = {}


def _build():
    if "nc" not in _CACHE:
        nc = bacc.Bacc("TRN2", target_bir_lowering=False, debug=False,
                       num_devices=NCORES)
        d = _declare(nc)
        with tile.TileContext(nc) as tc:
            _body(nc, tc, d)
        nc.compile()
        _CACHE["nc"] = nc
    return _CACHE["nc"]


def _q8(x):
    return np.clip(x, -240.0, 240.0).astype(f8)


def _feat_major(W):
    """[J, K] -> [128, K/128, J] (stationary lhsT chunk layout)."""
    J, K = W.shape
    return np.ascontiguousarray(
        W.reshape(J, K // 128, 128).transpose(2, 1, 0))


def _pack_inputs(inputs):
    z = np.asarray(inputs["z"], np.float32)
    dec_h0 = np.asarray(inputs["dec_h0"], np.float32)
    dec_c0 = np.asarray(inputs["dec_c0"], np.float32)
    cond_b = np.asarray(inputs["cond_bih"] + inputs["cond_bhh"], np.float32)
    dec_b = np.asarray(inputs["dec_bih"] + inputs["dec_bhh"], np.float32)
    out_b = np.asarray(inputs["out_b"], np.float32)

    shared = {
        "ones": np.ones((1, R), dtype=bf16),
        "cbias": np.ascontiguousarray(cond_b.reshape(G, 128).T).astype(np.float32),
        "dbias": np.ascontiguousarray(dec_b.reshape(G, 128).T).astype(np.float32),
        "obias": (WS * out_b)[None, :].astype(bf16),
        "cwih": _q8(WS * _feat_major(np.asarray(inputs["cond_Wih"], np.float32))),
        "cwhh": _q8(WS * _feat_major(np.asarray(inputs["cond_Whh"], np.float32))),
        "dwn": _q8(WS * _feat_major(np.asarray(inputs["dec_Wih"][:, H:], np.float32))),
        "dwhh": _q8(WS * _feat_major(np.asarray(inputs["dec_Whh"], np.float32))),
        "owt": _q8(WS * _feat_major(np.asarray(inputs["out_W"], np.float32))),
    }
    dwe_fm = _q8(WS * _feat_major(np.asarray(inputs["dec_Wih"][:, :H], np.float32)))
    # slab m: [128, HK*128] so each DMA is one contiguous read
    shared["dwe"] = np.ascontiguousarray(
        dwe_fm.reshape(128, HK, G, 128).transpose(2, 0, 1, 3).reshape(
            G, 128, HK * 128))

    z_lv = z[:, np.arange(L) * L, 0, :]           # [B, L, Z]
    in_maps = []
    for c in range(NCORES):
        bs = slice(c * Bc, (c + 1) * Bc)
        zc = z_lv[bs]                              # [Bc, L, Z]
        zT = _q8(np.ascontiguousarray(
            zc.reshape(Bc, L, ZK, 128).transpose(3, 2, 1, 0).reshape(128, ZK, R)))
        h0 = dec_h0[:, bs, :]                      # [L, Bc, H]
        h0T = np.ascontiguousarray(
            h0.reshape(L, Bc, HK, 128).transpose(3, 2, 0, 1).reshape(128, HK, R))
        c0 = dec_c0[:, bs, :]
        c0T = np.ascontiguousarray(
            c0.reshape(L, Bc, HK, 128).transpose(3, 2, 0, 1).reshape(128, HK, R))
        m = dict(shared)
        m["zT"] = zT
        m["h0T"] = _q8(h0T)
        m["c0T"] = c0T.astype(bf16)
        in_maps.append(m)
    return in_maps


def _unpack_outputs(core_outs):
    notes = np.empty((B, L * NS, T), np.float32)
    for c, arr in enumerate(core_outs):
        # arr [NS, TK, 128, R] -> [Bc, L, NS, T]
        a = arr.astype(np.float32).reshape(NS, TK, 128, L, Bc).transpose(4, 3, 0, 1, 2)
        notes[c * Bc:(c + 1) * Bc] = a.reshape(Bc, L, NS, T).reshape(
            Bc, L * NS, T)
    return notes


def kernel(**inputs):
    nc = _build()
    in_maps = _pack_inputs(inputs)
    res = run_bass_kernel_spmd(nc, in_maps, list(range(NCORES)))
    return _unpack_outputs([r["outbuf"] for r in res.results])


# revision 7
# speedup vs baseline: 2.7243x; 1.1264x over previous
"""Trainium2 Bass kernel for nn_Decoder (MusicVAE-style hierarchical decoder).

Strategy (8 NeuronCores, data-parallel over batch, no inter-core comms):
  - Conductor LSTM (16 sequential levels, batch 32/core) computes per-level
    embeddings.
  - Decoder levels are INDEPENDENT (initial state from dec_h0/dec_c0,
    note0=0), so all 16 levels are batched: effective decoder batch
    16*32 = 512 rows per core, 16 sequential note steps.
  - The conductor embedding is constant within a level, so its gate
    contribution ge = emb @ dec_Wih[:, :H].T + dec_b is precomputed once.
  - Everything feature-major: [features on partitions, rows free].
  - All matmuls run in fp8 e4m3 with DoubleRow perf mode (2x PE throughput,
    K=256 per instruction).  Weights are pre-scaled by WS=32 on the host to
    avoid the e4m3 denormal zone; the 1/WS dequant is folded into the
    scalar_tensor_tensor gate adds / activation scale (exact powers of 2).
  - PSUM gate layout [128, 4, R] with slot order (i, f, o, g) so one fused
    DVE scalar_tensor_tensor does psum/WS + ge, and one fused sigmoid covers
    i,f,o.  Elementwise work is split across DVE / Scalar / GPSIMD.
  - c state in bf16; h/note/emb in fp8 (matmul operands), output in bf16.
"""
import numpy as np
import ml_dtypes

import concourse.bacc as bacc
import concourse.tile as tile
import concourse.mybir as mybir
from concourse.bass_utils import run_bass_kernel_spmd

bf16 = ml_dtypes.bfloat16
f8 = ml_dtypes.float8_e4m3
F32 = mybir.dt.float32
BF = mybir.dt.bfloat16
F8 = mybir.dt.float8e4
AF = mybir.ActivationFunctionType
ALU = mybir.AluOpType
DR = mybir.MatmulPerfMode.DoubleRow

NCORES = 8
B, Z, H, T = 256, 512, 1024, 512
L, NS = 16, 16
Bc = B // NCORES            # 32 batch rows per core
R = L * Bc                  # 512 decoder rows per core (levels x batch)
HK, TK, ZK = H // 128, T // 128, Z // 128   # 8, 4, 4
G = 4 * H // 128            # 32 gate chunks of 128
HP, TP, ZP = HK // 2, TK // 2, ZK // 2      # k-pair counts (fp8 DoubleRow)
WS = 32.0                   # fp8 weight pre-scale
IVS = 1.0 / WS

# gate slot order per p: (i, f, o, g) -> column chunk in 4H
def _gate_ms(p):
    return (p, HK + p, 3 * HK + p, 2 * HK + p)


# slot index for gate-group g (0:i 1:f 2:g 3:o) in the (i,f,o,g) psum layout
_SLOT_OF_GROUP = {0: 0, 1: 1, 2: 3, 3: 2}


def _declare(nc):
    d = {}
    ei = dict(kind="ExternalInput")
    d["ones"] = nc.dram_tensor("ones", [1, R], BF, **ei)
    d["cbias"] = nc.dram_tensor("cbias", [128, G], F32, **ei)
    d["dbias"] = nc.dram_tensor("dbias", [128, G], F32, **ei)
    d["obias"] = nc.dram_tensor("obias", [1, T], BF, **ei)     # 32*out_b
    d["zT"] = nc.dram_tensor("zT", [128, ZK, R], F8, **ei)
    d["h0T"] = nc.dram_tensor("h0T", [128, HK, R], F8, **ei)
    d["c0T"] = nc.dram_tensor("c0T", [128, HK, R], BF, **ei)
    d["cwih"] = nc.dram_tensor("cwih", [128, ZK, 4 * H], F8, **ei)
    d["cwhh"] = nc.dram_tensor("cwhh", [128, HK, 4 * H], F8, **ei)
    d["dwe"] = nc.dram_tensor("dwe", [G, 128, HK * 128], F8, **ei)
    d["dwn"] = nc.dram_tensor("dwn", [128, TK, 4 * H], F8, **ei)
    d["dwhh"] = nc.dram_tensor("dwhh", [128, HK, 4 * H], F8, **ei)
    d["owt"] = nc.dram_tensor("owt", [128, HK, T], F8, **ei)
    d["outbuf"] = nc.dram_tensor("outbuf", [NS, TK, 128, R], BF,
                                 kind="ExternalOutput")
    return d


def _mm8(nc, out, w, x, start, stop):
    return nc.tensor.matmul(out, w, x, start=start, stop=stop, perf_mode=DR)


def _body(nc, tc, d):
    import contextlib
    with contextlib.ExitStack() as ctx:
        Pp = ctx.enter_context(tc.tile_pool(name="persist", bufs=1))

        t_ones = Pp.tile([1, R], BF, tag="ones")
        nc.sync.dma_start(t_ones[:], d["ones"][:])
        t_ob = Pp.tile([1, T], BF, tag="obias")
        nc.sync.dma_start(t_ob[:], d["obias"][:])
        t_emb8 = Pp.tile([128, HK, R], F8, tag="emb8")
        t_h = [Pp.tile([128, HK, R], F8, tag=f"hT{i}", name=f"hT{i}")
               for i in (0, 1)]
        t_c = Pp.tile([128, HK, R], BF, tag="c")
        t_note8 = Pp.tile([128, TK, R], F8, tag="note8")
        t_noteb = Pp.tile([128, TK, R], BF, tag="noteb")
        nc.gpsimd.dma_start(t_h[0][:], d["h0T"][:])
        nc.gpsimd.dma_start(t_c[:], d["c0T"][:])

        # ---------------- conductor ----------------
        with tc.tile_pool(name="cond", bufs=1) as Pc, \
             tc.tile_pool(name="ctmp", bufs=3) as Pt, \
             tc.tile_pool(name="cps", bufs=4, space="PSUM") as PSc, \
             tc.tile_pool(name="gzps", bufs=4, space="PSUM") as PSz:
            t_cb = Pc.tile([128, G], F32, tag="cbias")
            nc.sync.dma_start(t_cb[:], d["cbias"][:])
            t_cwih = Pc.tile([128, ZK, 4 * H], F8, tag="cwih")
            nc.sync.dma_start(t_cwih[:], d["cwih"][:])
            t_zT = Pc.tile([128, ZK, R], F8, tag="zT")
            nc.sync.dma_start(t_zT[:], d["zT"][:])
            t_cwhh = Pc.tile([128, HK, 4 * H], F8, tag="cwhh")
            nc.sync.dma_start(t_cwhh[:], d["cwhh"][:])
            # gz laid out p-major with slots (i,f,o,g), like the decoder
            t_gz = Pc.tile([128, HK, 4, R], BF, tag="gz")
            t_cc = Pc.tile([128, HK, Bc], F32, tag="cc")

            # gz = z @ cond_Wih.T + cond_b for all levels at once (fp8 DR)
            for m in range(G):
                ms = slice(m * 128, (m + 1) * 128)
                ps = PSz.tile([128, R], F32, tag="gzp")
                for j in range(ZP):
                    _mm8(nc, ps[:], t_cwih[:, 2 * j:2 * j + 2, ms],
                         t_zT[:, 2 * j:2 * j + 2, :], j == 0, j == ZP - 1)
                p_, s_ = m % HK, _SLOT_OF_GROUP[m // HK]
                if m % 2 == 0:
                    nc.vector.tensor_scalar(t_gz[:, p_, s_, :], ps[:], IVS,
                                            t_cb[:, m:m + 1],
                                            op0=ALU.mult, op1=ALU.add)
                else:
                    nc.scalar.activation(t_gz[:, p_, s_, :], ps[:],
                                         AF.Identity, bias=t_cb[:, m:m + 1],
                                         scale=IVS)

            # sequential levels
            for lv in range(L):
                cs = slice(lv * Bc, (lv + 1) * Bc)
                ps_prev = slice((lv - 1) * Bc, lv * Bc)
                for p in range(HK):
                    gms = _gate_ms(p)
                    if lv == 0:
                        # h0 == 0: gates are just gz; c0 == 0: c = sig(i)*tanh(g)
                        sio = Pt.tile([128, 3, Bc], BF, tag="csio")
                        tg = Pt.tile([128, Bc], BF, tag="ctg")
                        nc.scalar.activation(sio[:], t_gz[:, p, 0:3, cs],
                                             AF.Sigmoid)
                        nc.scalar.activation(tg[:], t_gz[:, p, 3, cs], AF.Tanh)
                        nc.vector.tensor_mul(t_cc[:, p, :], sio[:, 0, :], tg[:])
                    else:
                        ps = PSc.tile([128, 4, Bc], F32, tag="cgp")
                        for si in range(4):
                            ms = slice(gms[si] * 128, (gms[si] + 1) * 128)
                            for j in range(HP):
                                _mm8(nc, ps[:, si, :],
                                     t_cwhh[:, 2 * j:2 * j + 2, ms],
                                     t_emb8[:, 2 * j:2 * j + 2, ps_prev],
                                     j == 0, j == HP - 1)
                        gs = Pt.tile([128, 4, Bc], BF, tag="cgs")
                        nc.vector.scalar_tensor_tensor(
                            gs[:], ps[:], IVS, t_gz[:, p, :, cs],
                            op0=ALU.mult, op1=ALU.add)
                        sio = Pt.tile([128, 3, Bc], BF, tag="csio")
                        tg = Pt.tile([128, Bc], BF, tag="ctg")
                        nc.scalar.activation(sio[:], gs[:, 0:3, :], AF.Sigmoid)
                        nc.scalar.activation(tg[:], gs[:, 3, :], AF.Tanh)
                        tm1 = Pt.tile([128, Bc], BF, tag="ctm1")
                        tm2 = Pt.tile([128, Bc], F32, tag="ctm2")
                        nc.vector.tensor_mul(tm1[:], sio[:, 0, :], tg[:])
                        nc.vector.tensor_mul(tm2[:], sio[:, 1, :], t_cc[:, p, :])
                        nc.vector.tensor_add(t_cc[:, p, :], tm1[:], tm2[:])
                    tcn = Pt.tile([128, Bc], BF, tag="ctcn")
                    nc.scalar.activation(tcn[:], t_cc[:, p, :], AF.Tanh)
                    nc.vector.tensor_mul(t_emb8[:, p, cs], sio[:, 2, :], tcn[:])

        # ge persists through the decoder (allocated after conductor frees)
        Pge = ctx.enter_context(tc.tile_pool(name="gepool", bufs=1))
        t_ge = Pge.tile([128, HK, 4, R], BF, tag="ge")

        # decoder weights (bulk, SWDGE queues; overlap with ge phase)
        Pw = ctx.enter_context(tc.tile_pool(name="wdec", bufs=1))
        t_dwn = Pw.tile([128, TK, 4 * H], F8, tag="dwn")
        nc.gpsimd.dma_start(t_dwn[:], d["dwn"][:])
        t_dwhh = Pw.tile([128, HK, 4 * H], F8, tag="dwhh")
        nc.gpsimd.dma_start(t_dwhh[:], d["dwhh"][:])
        t_owt = Pw.tile([128, HK, T], F8, tag="owt")
        nc.gpsimd.dma_start(t_owt[:], d["owt"][:])

        # ---------------- ge = emb @ dec_Wih[:, :H].T + dec_b ----------------
        with tc.tile_pool(name="gew", bufs=4) as Pgw, \
             tc.tile_pool(name="geps", bufs=4, space="PSUM") as PSg:
            t_db = Pgw.tile([128, G], F32, tag="dbias", bufs=1)
            nc.sync.dma_start(t_db[:], d["dbias"][:])
            for m in range(G):
                wt = Pgw.tile([128, HK, 128], F8, tag="dwe")
                nc.sync.dma_start(wt[:], d["dwe"][m].rearrange(
                    "p (k j) -> p k j", k=HK))
                ps = PSg.tile([128, R], F32, tag="gep")
                for j in range(HP):
                    _mm8(nc, ps[:], wt[:, 2 * j:2 * j + 2, :],
                         t_emb8[:, 2 * j:2 * j + 2, :], j == 0, j == HP - 1)
                p_, s_ = m % HK, _SLOT_OF_GROUP[m // HK]
                if m % 2 == 0:
                    nc.vector.tensor_scalar(t_ge[:, p_, s_, :], ps[:], IVS,
                                            t_db[:, m:m + 1],
                                            op0=ALU.mult, op1=ALU.add)
                else:
                    nc.scalar.activation(t_ge[:, p_, s_, :], ps[:],
                                         AF.Identity, bias=t_db[:, m:m + 1],
                                         scale=IVS)

        # ---------------- decoder: 16 note steps over 512 rows --------------
        # The 16 levels split into two independent row-cohorts (RC=256 rows
        # each) whose steps interleave: cohort A's serial drain/oproj tail
        # hides under cohort B's matmul phase.  Within a cohort-step the
        # emission is software-pipelined (stage A at p, stage B at p-1) so
        # each engine's in-order queue never waits mid-chain.
        RC = R // 2
        with tc.tile_pool(name="dtmp", bufs=3) as Pdt, \
             tc.tile_pool(name="dps", bufs=4, space="PSUM") as PSd:
            stage = {}

            def stage_a(rs, pp, gs):
                # activations for the p-pair (2pp, 2pp+1) fused
                sio = Pdt.tile([128, 2, 3, RC], BF, tag="sio")
                tg = Pdt.tile([128, 2, RC], BF, tag="tg")
                nc.scalar.activation(sio[:], gs[:, :, 0:3, :], AF.Sigmoid)
                nc.scalar.activation(tg[:], gs[:, :, 3, :], AF.Tanh)
                tm1 = Pdt.tile([128, 2, RC], BF, tag="tm1")
                nc.gpsimd.tensor_mul(tm1[:], sio[:, :, 0, :], tg[:])
                stage[pp] = (sio, tm1)

            def stage_b(rs, pp, hout):
                sio, tm1 = stage.pop(pp)
                ps = slice(2 * pp, 2 * pp + 2)
                tm2 = Pdt.tile([128, 2, RC], BF, tag="tm2")
                tcn = Pdt.tile([128, 2, RC], BF, tag="tcn")
                nc.vector.tensor_mul(tm2[:], sio[:, :, 1, :], t_c[:, ps, rs])
                nc.vector.tensor_add(t_c[:, ps, rs], tm1[:], tm2[:])
                nc.scalar.activation(tcn[:], t_c[:, ps, rs], AF.Tanh)
                nc.gpsimd.tensor_mul(hout[:, ps, rs], sio[:, :, 2, :], tcn[:])

            def cohort_step(t, rh):
                rs = slice(rh * RC, (rh + 1) * RC)
                hin = t_h[t % 2]
                hout = t_h[(t + 1) % 2]
                for pp in range(HK // 2):
                    gs = Pdt.tile([128, 2, 4, RC], BF, tag="gs")
                    for pi in range(2):
                        p = 2 * pp + pi
                        gms = _gate_ms(p)
                        pt = PSd.tile([128, 4, RC], F32, tag="dgp")
                        for si in range(4):
                            ms = slice(gms[si] * 128, (gms[si] + 1) * 128)
                            if t > 0:
                                # note contribution first: note8(t-1) is the
                                # freshest input, h pairs follow
                                for j in range(TP):
                                    _mm8(nc, pt[:, si, :],
                                         t_dwn[:, 2 * j:2 * j + 2, ms],
                                         t_note8[:, 2 * j:2 * j + 2, rs],
                                         j == 0, False)
                            for j in range(HP):
                                _mm8(nc, pt[:, si, :],
                                     t_dwhh[:, 2 * j:2 * j + 2, ms],
                                     hin[:, 2 * j:2 * j + 2, rs],
                                     (j == 0 and t == 0), j == HP - 1)
                        nc.vector.scalar_tensor_tensor(
                            gs[:, pi, :, :], pt[:], IVS, t_ge[:, p, :, rs],
                            op0=ALU.mult, op1=ALU.add)
                    stage_a(rs, pp, gs)
                    if pp >= 1:
                        stage_b(rs, pp - 1, hout)
                stage_b(rs, HK // 2 - 1, hout)
                # output projection + sigmoid -> bf16 out + fp8 feedback copy
                po = PSd.tile([128, TK, RC], F32, tag="dgp", name="po")
                for tk in range(TK):
                    ts_ = slice(tk * 128, (tk + 1) * 128)
                    nc.tensor.matmul(po[:, tk, :], t_ob[0:1, ts_],
                                     t_ones[0:1, rs], start=True, stop=False)
                    for j in range(HP):
                        _mm8(nc, po[:, tk, :], t_owt[:, 2 * j:2 * j + 2, ts_],
                             hout[:, 2 * j:2 * j + 2, rs], False, j == HP - 1)
                nc.scalar.activation(t_noteb[:, :, rs], po[:], AF.Sigmoid,
                                     scale=IVS)
                if t < NS - 1:
                    nc.vector.tensor_copy(t_note8[:, :, rs],
                                          t_noteb[:, :, rs])
                for tk in range(TK):
                    nc.sync.dma_start(d["outbuf"][t, tk, :, rs],
                                      t_noteb[:, tk, rs])

            for t in range(NS):
                cohort_step(t, 0)
                cohort_step(t, 1)


_CACHE = {}


def _build():
    if "nc" not in _CACHE:
        nc = bacc.Bacc("TRN2", target_bir_lowering=False, debug=False,
                       num_devices=NCORES)
        d = _declare(nc)
        with tile.TileContext(nc) as tc:
            _body(nc, tc, d)
        nc.compile()
        _CACHE["nc"] = nc
    return _CACHE["nc"]


def _q8(x):
    return np.clip(x, -240.0, 240.0).astype(f8)


def _feat_major(W):
    """[J, K] -> [128, K/128, J] (stationary lhsT chunk layout)."""
    J, K = W.shape
    return np.ascontiguousarray(
        W.reshape(J, K // 128, 128).transpose(2, 1, 0))


def _pack_inputs(inputs):
    z = np.asarray(inputs["z"], np.float32)
    dec_h0 = np.asarray(inputs["dec_h0"], np.float32)
    dec_c0 = np.asarray(inputs["dec_c0"], np.float32)
    cond_b = np.asarray(inputs["cond_bih"] + inputs["cond_bhh"], np.float32)
    dec_b = np.asarray(inputs["dec_bih"] + inputs["dec_bhh"], np.float32)
    out_b = np.asarray(inputs["out_b"], np.float32)

    shared = {
        "ones": np.ones((1, R), dtype=bf16),
        "cbias": np.ascontiguousarray(cond_b.reshape(G, 128).T).astype(np.float32),
        "dbias": np.ascontiguousarray(dec_b.reshape(G, 128).T).astype(np.float32),
        "obias": (WS * out_b)[None, :].astype(bf16),
        "cwih": _q8(WS * _feat_major(np.asarray(inputs["cond_Wih"], np.float32))),
        "cwhh": _q8(WS * _feat_major(np.asarray(inputs["cond_Whh"], np.float32))),
        "dwn": _q8(WS * _feat_major(np.asarray(inputs["dec_Wih"][:, H:], np.float32))),
        "dwhh": _q8(WS * _feat_major(np.asarray(inputs["dec_Whh"], np.float32))),
        "owt": _q8(WS * _feat_major(np.asarray(inputs["out_W"], np.float32))),
    }
    dwe_fm = _q8(WS * _feat_major(np.asarray(inputs["dec_Wih"][:, :H], np.float32)))
    # slab m: [128, HK*128] so each DMA is one contiguous read
    shared["dwe"] = np.ascontiguousarray(
        dwe_fm.reshape(128, HK, G, 128).transpose(2, 0, 1, 3).reshape(
            G, 128, HK * 128))

    z_lv = z[:, np.arange(L) * L, 0, :]           # [B, L, Z]
    in_maps = []
    for c in range(NCORES):
        bs = slice(c * Bc, (c + 1) * Bc)
        zc = z_lv[bs]                              # [Bc, L, Z]
        zT = _q8(np.ascontiguousarray(
            zc.reshape(Bc, L, ZK, 128).transpose(3, 2, 1, 0).reshape(128, ZK, R)))
        h0 = dec_h0[:, bs, :]                      # [L, Bc, H]
        h0T = np.ascontiguousarray(
            h0.reshape(L, Bc, HK, 128).transpose(3, 2, 0, 1).reshape(128, HK, R))
        c0 = dec_c0[:, bs, :]
        c0T = np.ascontiguousarray(
            c0.reshape(L, Bc, HK, 128).transpose(3, 2, 0, 1).reshape(128, HK, R))
        m = dict(shared)
        m["zT"] = zT
        m["h0T"] = _q8(h0T)
        m["c0T"] = c0T.astype(bf16)
        in_maps.append(m)
    return in_maps


def _unpack_outputs(core_outs):
    notes = np.empty((B, L * NS, T), np.float32)
    for c, arr in enumerate(core_outs):
        # arr [NS, TK, 128, R] -> [Bc, L, NS, T]
        a = arr.astype(np.float32).reshape(NS, TK, 128, L, Bc).transpose(4, 3, 0, 1, 2)
        notes[c * Bc:(c + 1) * Bc] = a.reshape(Bc, L, NS, T).reshape(
            Bc, L * NS, T)
    return notes


def kernel(**inputs):
    nc = _build()
    in_maps = _pack_inputs(inputs)
    res = run_bass_kernel_spmd(nc, in_maps, list(range(NCORES)))
    return _unpack_outputs([r["outbuf"] for r in res.results])
